# revision 73
# baseline (speedup 1.0000x reference)
"""Trainium2 Bass kernel for nn_DrugResponsePrior (embedding_lookup).

Spec guarantees: cell_map < 100, is_missing in {0,1}, drug_map < 256.  Each
row's result depends only on the cell state cs = cell_map[idx] +
100*is_missing[idx] (200 states) and dm = drug_map[tidx] (256 drugs).

Fully data-parallel (8 cores x 8192 samples, no collectives).  Per core:
  1. Host bit-packs csmi = cell_map | (is_missing << 7) (u8 - a pure bit
     repack; the state CODE cm + 128*mi is exact in bf16).  csmi/drug_map are
     loaded in a 16-slab SBUF layout (partition 16g+r holds entries
     [r*16384, (r+1)*16384) for every group g).
  2. Two GPSIMD indirect_copy gathers per 4096-sample piece fetch the 16
     slab candidates per sample; a one-hot mask over idx>>14 (grp_bc matmul +
     is_eq) and a group-reduce matmul produce v8 = per-sample code [8, 1024].
  3. Tables built once on device: A = l2n(cell emb) @ Wf1c + bf1 ([200,200])
     and Bd = l2n(drug_emb) @ Wf1d ([256,200]), bf16.
  4. Per 512-sample chunk: gpsimd partition_broadcast expands the codes to
     [128, 512]; two DVE is_eq (4x mode, bf16) build one-hot matrices; bf16
     matmuls run the MLP: h1 = relu(A^T Sc + Bd^T Sd), h2 = relu(Wf2^T h1 +
     bf2), fm = [fwd(1:9); mu-base] via one packed [.,17] lhsT, softplus on
     scalar engine, one L8 matmul accumulates the cumsum into the mu rows.
  Chunks are software-pipelined depth 3 so every engine streams without
  gaps (keeps the PE p-state at full clock).

All params ride in two packed blob tensors ([128, N] with large contiguous
partition lines) so the whole param load is 2 DMAs - the baseline's ~16k
small DMA descriptors were the main bottleneck.

All reference math runs on device; the host only reshapes/transposes/casts
inputs, bit-packs the two sub-byte index tables, and slices idx/tidx (pure
index arithmetic: & 16383, >> 14).
"""
import os
import sys

if "/opt/trn_rl_repo" not in sys.path:
    sys.path.insert(0, "/opt/trn_rl_repo")

# Neither CoreSim nor this neuronxcc's activation tables implement Softplus;
# softplus is computed as ln(1 + exp(x)) in two scalar passes.
USE_SOFTPLUS = False

import numpy as np
import ml_dtypes

import concourse.bass as bass
import concourse.bass_isa as bass_isa
import concourse.mybir as mybir
import concourse.tile as tile
from concourse.bass_utils import run_bass_kernel_spmd

f32 = mybir.dt.float32
bf16 = mybir.dt.bfloat16
u16 = mybir.dt.uint16
u8 = mybir.dt.uint8
np_bf16 = ml_dtypes.bfloat16

B = 65536
R = 262144
NDRUG = 256
NFEAT = 1024
CEMB = 1024
DEMB = 128
HID = 200
NDOSES = 9
NCORES = 8

BS = B // NCORES            # 8192 samples per core
P = 128
NG = 8                      # gpsimd groups (16 partitions each)
GS = BS // NG               # 1024 samples per group
SLAB = R // 16              # 16384 entries per slab partition
EPS = 1e-12

_NC_CACHE = {}

# ---------------- packed blob layouts (host & device share these) ----------
def _layout(specs):
    out, off = {}, 0
    for name, n in specs:
        out[name] = (off, off + n)
        off += n
    return out, off

# bf16 param blobs, split by when the device needs them (3 staged DMAs)
T1_L, NT1 = _layout([
    ("w1", 8 * CEMB),        # 8 k-tiles [128, 1024]
    ("cft", 8 * 100),        # 8 k-tiles [128, 100]
])
T2_L, NT2 = _layout([
    ("wf1c", 8 * HID),       # 8 k-tiles [128, 200]
    ("wf1d", HID),           # [128, 200]
    ("deT", NDRUG),          # [128, 256] drug_emb^T
    ("meb", CEMB),           # rows 0:100 = missing_emb
    ("de", 2 * DEMB),        # drug_emb [256, 128] as two [128, 128] tiles
])
# tiny early-loaded bf16 constants (lookup resolve must not wait for t2)
TS_L, NTS = _layout([
    ("grp_bc", P),           # rows 0:8: [g, p] = (p//16 == g)
    ("grp_rd", NG),          # [p, g] = (p//16 == g)
    ("selg", NG * P),        # block g: [g', p] = (g' == g)  (bcast lhsT)
])
T3_L, NT3 = _layout([
    ("wf2a", HID),           # Wf2[0:128, :]
    ("wf2b", HID),           # rows 0:72 = Wf2[128:200, :]
    ("fma", 17),             # [Wf3p[0:128, 0:8] | tile(col base, 9)]
    ("fmb", 17),             # rows 0:72 = Wf3p[128:200]; row 127 = biases
    ("l8", 17),              # rows 0:8: cols 0:8 zero, cols 8:17 (k < o)
    ("ones512", 512),        # row 0 = 1.0 (DMA'd to h2s1_st row 127)
])
# single-row f32 tensor (bias rows for matmul rhs; base partition 0)
BR_L, NBR = _layout([
    ("b1r", CEMB),
    ("bf1r", HID),
    ("onesr", P),
])
# per-partition f32 columns [128, NBC]
BC_L, NBC = _layout([
    ("qi", 1),               # p % 16
    ("ccl", 1),              # cs code, one-hot block lo
    ("cch", 1),              # cs code, block hi
    ("cdl", 1),              # dm code lo
    ("cdh", 1),              # dm code hi
    ("bf2a", 1),
    ("bf2b", 1),
])


def _split_sync_waits(nc, limit=1):
    """The walrus accepts at most one sync-wait per instruction; hoist excess
    waits onto same-engine NoOps inserted just before."""
    ctr = 0
    for bb in nc.main_func.blocks:
        new_list = []
        for inst in bb.instructions:
            si = inst.sync_info
            if si is not None and si.on_wait and len(si.on_wait) > limit:
                waits = list(si.on_wait)
                head, tail = waits[:-limit], waits[-limit:]
                for j in range(0, len(head), limit):
                    nop = mybir.InstNoOp(name=f"waitnop-{ctr}", engine=inst.engine)
                    ctr += 1
                    nop.sync_info = mybir.SyncInfo(
                        on_wait=list(head[j : j + limit]), on_update=[]
                    )
                    new_list.append(nop)
                inst.sync_info = mybir.SyncInfo(
                    on_wait=list(tail),
                    on_update=list(si.on_update) if si.on_update else [],
                )
            new_list.append(inst)
        bb.instructions[:] = new_list
    return nc


def build_nc(split_waits=True):
    nc = bass.Bass(num_devices=NCORES)
    AF = mybir.ActivationFunctionType
    ALU = mybir.AluOpType

    # ---------------- kernel I/O ----------------
    blob_t1 = nc.dram_tensor("blob_t1", [P, NT1], bf16, kind="ExternalInput")
    blob_t2 = nc.dram_tensor("blob_t2", [P, NT2], bf16, kind="ExternalInput")
    blob_t3 = nc.dram_tensor("blob_t3", [P, NT3], bf16, kind="ExternalInput")
    blob_ts = nc.dram_tensor("blob_ts", [P, NTS], bf16, kind="ExternalInput")
    brow = nc.dram_tensor("brow", [1, NBR], f32, kind="ExternalInput")
    bcol = nc.dram_tensor("bcol", [P, NBC], f32, kind="ExternalInput")
    # host pre-replicates the 16-slab tables to all 8 groups (contiguous
    # DMA: ~266 GB/s vs ~80 GB/s for a stride-0 replicating read)
    csmi = nc.dram_tensor("csmi", [P, SLAB], u8, kind="ExternalInput")
    dm_map = nc.dram_tensor("dm_map", [P, SLAB], u8, kind="ExternalInput")
    u_idx = nc.dram_tensor("u_idx", [P, GS // 16], u16, kind="ExternalInput")
    u_tidx = nc.dram_tensor("u_tidx", [P, GS // 16], u16, kind="ExternalInput")
    q_idx = nc.dram_tensor("q_idx", [NG, GS], bf16, kind="ExternalInput")
    q_tidx = nc.dram_tensor("q_tidx", [NG, GS], bf16, kind="ExternalInput")
    mu9_s = nc.dram_tensor("mu9_s", [NDOSES, BS], f32, kind="ExternalOutput")
    # looked-up codes bounce through DRAM so chunks can broadcast-load them
    # with stride-0 partition reads (not expressible from SBUF)
    v8d_cs = nc.dram_tensor("v8d_cs", [NG * GS], bf16)
    v8d_dm = nc.dram_tensor("v8d_dm", [NG * GS], bf16)

    with tile.TileContext(nc) as tc, \
            tc.tile_pool(name="sbw", bufs=1) as sbw, \
            tc.tile_pool(name="sb", bufs=1) as sb:

        # ---- setup DMAs ----
        # queue A (sync -> DMA engines 0-7): index tensors + slabs
        # queue B (scalar -> DMA engines 8-15): param blobs, staged
        u_idx_sb = sb.tile([P, GS // 16], u16)
        u_tidx_sb = sb.tile([P, GS // 16], u16)
        q_idx_sb = sb.tile([NG, GS], bf16)
        q_tidx_sb = sb.tile([NG, GS], bf16)
        bc_sb = sb.tile([P, NBC], f32)
        nc.sync.dma_start(out=bc_sb[:], in_=bcol[:])
        nc.sync.dma_start(out=u_idx_sb[:], in_=u_idx[:])
        nc.sync.dma_start(out=u_tidx_sb[:], in_=u_tidx[:])
        nc.sync.dma_start(out=q_idx_sb[:], in_=q_idx[:])
        nc.sync.dma_start(out=q_tidx_sb[:], in_=q_tidx[:])
        cs_slab = sbw.tile([P, SLAB], u8)
        dm_slab = sbw.tile([P, SLAB], u8)
        t1_sb = sbw.tile([P, NT1], bf16)
        # slabs split across both queue groups: cs halves first (the gather
        # train gates everything downstream), then dm halves
        br_sb = sb.tile([1, NBR], f32)
        nc.scalar.dma_start(out=br_sb[:], in_=brow[:])
        ts_sb = sb.tile([P, NTS], bf16)
        nc.scalar.dma_start(out=ts_sb[:], in_=blob_ts[:])
        nc.sync.dma_start(out=cs_slab[0:64, :], in_=csmi[0:64, :])
        nc.scalar.dma_start(out=cs_slab[64:P, :], in_=csmi[64:P, :])
        nc.sync.dma_start(out=dm_slab[0:64, :], in_=dm_map[0:64, :])
        nc.scalar.dma_start(out=dm_slab[64:P, :], in_=dm_map[64:P, :])
        # t1 streamed per k-tile, alternating queues, so the P100 matmuls
        # start as soon as each tile lands instead of after the whole blob
        for kt in range(8):
            eng = nc.sync if kt % 2 == 0 else nc.scalar
            w0, _ = T1_L["w1"]
            c0, _ = T1_L["cft"]
            eng.dma_start(out=t1_sb[:, w0 + kt * CEMB:w0 + (kt + 1) * CEMB],
                          in_=blob_t1[:, w0 + kt * CEMB:w0 + (kt + 1) * CEMB])
            eng.dma_start(out=t1_sb[:, c0 + kt * 100:c0 + (kt + 1) * 100],
                          in_=blob_t1[:, c0 + kt * 100:c0 + (kt + 1) * 100])
        t2_sb = sbw.tile([P, NT2], bf16)
        nc.scalar.dma_start(out=t2_sb[:], in_=blob_t2[:])
        t3_sb = sbw.tile([P, NT3], bf16)
        nc.scalar.dma_start(out=t3_sb[:], in_=blob_t3[:])

        # blob views
        me_sb = t2_sb[0:100, T2_L["meb"][0]:T2_L["meb"][1]]
        b1_row = br_sb[:, BR_L["b1r"][0]:BR_L["b1r"][1]]
        bf1_row = br_sb[:, BR_L["bf1r"][0]:BR_L["bf1r"][1]]
        ones100 = br_sb[:, BR_L["onesr"][0]:BR_L["onesr"][0] + 100]
        ones128 = br_sb[:, BR_L["onesr"][0]:BR_L["onesr"][1]]
        de0 = t2_sb[:, T2_L["de"][0]:T2_L["de"][0] + DEMB]
        de1 = t2_sb[:, T2_L["de"][0] + DEMB:T2_L["de"][0] + 2 * DEMB]
        qi_c = bc_sb[:, BC_L["qi"][0]:BC_L["qi"][1]]
        ccl_c = bc_sb[:, BC_L["ccl"][0]:BC_L["ccl"][1]]
        cch_c = bc_sb[:, BC_L["cch"][0]:BC_L["cch"][1]]
        cdl_c = bc_sb[:, BC_L["cdl"][0]:BC_L["cdl"][1]]
        cdh_c = bc_sb[:, BC_L["cdh"][0]:BC_L["cdh"][1]]
        bf2a_c = bc_sb[:, BC_L["bf2a"][0]:BC_L["bf2a"][1]]
        bf2b_c = bc_sb[0:72, BC_L["bf2b"][0]:BC_L["bf2b"][1]]
        w1_kt = [t1_sb[:, T1_L["w1"][0] + k * CEMB:T1_L["w1"][0] + (k + 1) * CEMB]
                 for k in range(8)]
        cft_kt = [t1_sb[:, T1_L["cft"][0] + k * 100:T1_L["cft"][0] + (k + 1) * 100]
                  for k in range(8)]
        wf1c_kt = [t2_sb[:, T2_L["wf1c"][0] + k * HID:T2_L["wf1c"][0] + (k + 1) * HID]
                   for k in range(8)]
        wf1d_sb = t2_sb[:, T2_L["wf1d"][0]:T2_L["wf1d"][1]]
        deT_sb = t2_sb[:, T2_L["deT"][0]:T2_L["deT"][1]]
        grp_bc = ts_sb[0:NG, TS_L["grp_bc"][0]:TS_L["grp_bc"][1]]
        grp_rd = ts_sb[:, TS_L["grp_rd"][0]:TS_L["grp_rd"][1]]
        selg = [ts_sb[0:NG, TS_L["selg"][0] + g * P:TS_L["selg"][0] + (g + 1) * P]
                for g in range(NG)]
        wf2a = t3_sb[:, T3_L["wf2a"][0]:T3_L["wf2a"][1]]
        wf2b = t3_sb[0:72, T3_L["wf2b"][0]:T3_L["wf2b"][1]]
        fma = t3_sb[:, T3_L["fma"][0]:T3_L["fma"][1]]
        fmb = t3_sb[:, T3_L["fmb"][0]:T3_L["fmb"][1]]
        l8_sb = t3_sb[0:8, T3_L["l8"][0]:T3_L["l8"][1]]

        # static h2 tiles (relu outputs; h2s1 rows 72:127 zero, row 127 ones
        # so fmb's row 127 supplies the biases)
        h2s0_st = sbw.tile([P, 512], bf16)
        h2s1_st = sbw.tile([P, 512], bf16)
        nc.vector.memset(h2s1_st[:], 0.0)
        nc.sync.dma_start(
            out=h2s1_st[P - 1:P, :],
            in_=blob_t3[0:1, T3_L["ones512"][0]:T3_L["ones512"][1]])

        # lookup state (lives across the table scope and the chunk scope)
        g_cs = sb.tile([P, GS], u8)
        g_dm = sb.tile([P, GS], u8)
        v8_cs = sb.tile([NG, GS], bf16)
        v8_dm = sb.tile([NG, GS], bf16)

        def emit_resolve(t, mk_ps, names=("c", "d")):
            jsl = slice(t * 512, (t + 1) * 512)
            for (gt, qt, v8t, nm) in ((g_cs, q_idx_sb, v8_cs, "c"),
                                      (g_dm, q_tidx_sb, v8_dm, "d")):
                if nm not in names:
                    continue
                qb = mk_ps()
                nc.tensor.matmul(out=qb[:], lhsT=grp_bc, rhs=qt[:, jsl],
                                 start=True, stop=True)
                qmask = sb.tile([P, 512], bf16, tag=f"qmask_{nm}",
                                name=f"qmask_{nm}")
                nc.vector.tensor_scalar(
                    out=qmask[:], in0=qb[:], scalar1=qi_c, scalar2=None,
                    op0=ALU.is_equal)
                gf = sb.tile([P, 512], bf16, tag=f"gf_{nm}", name=f"gf_{nm}")
                nc.vector.tensor_copy(out=gf[:], in_=gt[:, jsl])
                nc.vector.tensor_tensor(out=gf[:], in0=gf[:], in1=qmask[:],
                                        op=ALU.mult)
                vpf = mk_ps()
                nc.tensor.matmul(out=vpf[0:NG, :], lhsT=grp_rd, rhs=gf[:],
                                 start=True, stop=True)
                nc.vector.tensor_copy(out=v8t[:, jsl], in_=vpf[0:NG, :])
                v8dt = v8d_cs if nm == "c" else v8d_dm
                nc.sync.dma_start(
                    out=v8dt[:].rearrange("(g j) -> g j", g=NG)[:, jsl],
                    in_=v8t[:, jsl])

        # ======== table construction: A [200,200], Bd [256,200] (bf16) ======
        a_k = []
        bd_k = []
        with (
            tc.tile_pool(name="ps_tb", bufs=1, space="PSUM") as ps_tb,
            tc.tile_pool(name="ps_tr", bufs=3, space="PSUM") as ps_tr,
            tc.tile_pool(name="sbt", bufs=1) as sbt,
        ):
            from concourse.masks import make_identity
            ident = sbt.tile([P, P], bf16)
            make_identity(nc, ident[:])

            # P100 = relu(cf @ W1 + b1)  [100, 1024]; kt-major so each matmul
            # runs as soon as its streamed k-tile DMA lands
            p_sb = sbt.tile([100, CEMB], bf16)
            pps = [ps_tb.tile([100, 512], f32, tag=f"pshard{nh}",
                              name=f"pps{nh}") for nh in range(2)]
            for kt in range(8):
                for nh in range(2):
                    nc.tensor.matmul(
                        out=pps[nh][:], lhsT=cft_kt[kt],
                        rhs=w1_kt[kt][:, nh * 512:(nh + 1) * 512],
                        start=(kt == 0), stop=False)
            for nh in range(2):
                nc.tensor.matmul(
                    out=pps[nh][:], lhsT=ones100,
                    rhs=b1_row[:, nh * 512:(nh + 1) * 512], start=False, stop=True)
                nc.scalar.activation(
                    out=p_sb[:, nh * 512:(nh + 1) * 512], in_=pps[nh][:],
                    func=AF.Relu)

            # l2 norm scales for present / missing rows
            sq = sbt.tile([100, CEMB], f32)
            ssp = sbt.tile([100, 1], f32)
            ssm = sbt.tile([100, 1], f32)
            nc.scalar.activation(out=sq[:], in_=p_sb[:], func=AF.Square)
            nc.vector.reduce_sum(out=ssp[:], in_=sq[:], axis=mybir.AxisListType.X)
            nc.scalar.activation(out=sq[:], in_=me_sb, func=AF.Square)
            nc.vector.reduce_sum(out=ssm[:], in_=sq[:], axis=mybir.AxisListType.X)
            for ss in (ssp, ssm):
                nc.scalar.activation(out=ss[:], in_=ss[:], func=AF.Sqrt)
                nc.vector.tensor_scalar_max(out=ss[:], in0=ss[:], scalar1=EPS)
                nc.vector.reciprocal(out=ss[:], in_=ss[:])
            nc.vector.tensor_scalar_mul(out=p_sb[:], in0=p_sb[:], scalar1=ssp[:])
            nc.vector.tensor_scalar_mul(out=me_sb, in0=me_sb, scalar1=ssm[:])

            # CnT k-tiles [128, 200] bf16 (cols: 100 present + 100 missing)
            cnt_kt = []
            for kt in range(8):
                t = sbt.tile([P, 2 * 100], bf16, tag=f"cnt_{kt}")
                for (src, co) in ((p_sb[:], 0), (me_sb, 100)):
                    tp = ps_tr.tile([P, 100], bf16, tag="tr")
                    nc.tensor.transpose(
                        out=tp[:], in_=src[:, kt * P:(kt + 1) * P],
                        identity=ident[:100, :100])
                    nc.vector.tensor_copy(out=t[:, co:co + 100], in_=tp[:])
                cnt_kt.append(t)

            # A tiles (states on partitions): a_k[0] [128, 200], a_k[1] [72, 200]
            for (mt, msl) in ((0, slice(0, P)), (1, slice(P, HID))):
                mm = msl.stop - msl.start
                aps = ps_tb.tile([P, HID], f32, tag="a")
                for kt in range(8):
                    nc.tensor.matmul(
                        out=aps[:mm, :], lhsT=cnt_kt[kt][:, msl],
                        rhs=wf1c_kt[kt], start=(kt == 0), stop=False)
                nc.tensor.matmul(
                    out=aps[:mm, :], lhsT=ones128[:, :mm], rhs=bf1_row,
                    start=False, stop=True)
                t = sb.tile([mm, HID], bf16, tag=f"a_{mt}")
                nc.vector.tensor_copy(out=t[:], in_=aps[:mm, :])
                a_k.append(t)

            # drug tiles: per-drug l2 recip + Bd [128, 200] bf16 x2
            for (mt, de_p) in ((0, de0), (1, de1)):
                sqd = sbt.tile([P, DEMB], f32, tag="sqd")
                rd = sbt.tile([P, 1], f32, tag=f"rd_{mt}")
                nc.scalar.activation(out=sqd[:], in_=de_p, func=AF.Square)
                nc.vector.reduce_sum(out=rd[:], in_=sqd[:], axis=mybir.AxisListType.X)
                nc.scalar.activation(out=rd[:], in_=rd[:], func=AF.Sqrt)
                nc.vector.tensor_scalar_max(out=rd[:], in0=rd[:], scalar1=EPS)
                nc.vector.reciprocal(out=rd[:], in_=rd[:])
                bps = ps_tb.tile([P, HID], f32, tag="a")
                nc.tensor.matmul(out=bps[:], lhsT=deT_sb[:, mt * P:(mt + 1) * P],
                                 rhs=wf1d_sb, start=True, stop=True)
                t = sb.tile([P, HID], bf16, tag=f"bd_{mt}")
                nc.vector.tensor_scalar_mul(out=t[:], in0=bps[:], scalar1=rd[:])
                bd_k.append(t)

            # ======== lookup gathers (gpsimd runs these back to back) ========
            for t in range(2):
                nc.gpsimd.indirect_copy(
                    out=g_cs[:, t * 512:(t + 1) * 512].rearrange(
                        "p (n one) -> p n one", one=1),
                    data=cs_slab[:], idxs=u_idx_sb[:, t * 32:(t + 1) * 32],
                    i_know_ap_gather_is_preferred=True)
                nc.gpsimd.indirect_copy(
                    out=g_dm[:, t * 512:(t + 1) * 512].rearrange(
                        "p (n one) -> p n one", one=1),
                    data=dm_slab[:], idxs=u_tidx_sb[:, t * 32:(t + 1) * 32],
                    i_know_ap_gather_is_preferred=True)
            # piece 0 resolved here; piece 1 resolved mid-chunk-stream so the
            # engine FIFOs don't head-of-line block on its gathers
            emit_resolve(0, lambda: ps_tb.tile([P, 512], f32, tag="pqb",
                                               name="pqb"))

        # ======== per-chunk pipeline ========
        chunks = [(g, pc) for pc in range(2) for g in range(NG)]
        NCH = len(chunks)

        with (
            tc.tile_pool(name="ps_h1", bufs=2, space="PSUM") as ps_h1,
            tc.tile_pool(name="ps_h2", bufs=1, space="PSUM") as ps_h2,
            tc.tile_pool(name="ps_fm", bufs=2, space="PSUM") as ps_fm,
            tc.tile_pool(name="sbc", bufs=2) as sbc,
        ):
            bc_of, oh_of, h1ps_of, h1s_of, h2ps_of, fm_of, spb_of = \
                {}, {}, {}, {}, {}, {}, {}

            def emit_qb(i):
                # broadcast codes of chunk i's group to all 128 partitions
                # via SBUF->SBUF DMA with a stride-0 partition source (the
                # DMA queues are idle during the chunk phase)
                g, pc = chunks[i]
                bcc = sbc.tile([P, 512], bf16, tag="bcc")
                bcd = sbc.tile([P, 512], bf16, tag="bcd")
                for (bc, v8dt) in ((bcc, v8d_cs), (bcd, v8d_dm)):
                    nc.sync.dma_start(out=bc[:], in_=bass.AP(
                        tensor=v8dt.ap().tensor, offset=g * GS + pc * 512,
                        ap=[[0, P], [1, 512]]))
                bc_of[i] = (bcc, bcd)

            def emit_bcopy(i):
                pass

            def emit_onehot(i):
                eng = nc.vector
                bcc, bcd = bc_of.pop(i)
                sc2 = sbc.tile([P, 1024], bf16, tag="sc2")
                sd2 = sbc.tile([P, 1024], bf16, tag="sd2")
                for (oh, bc, cl, ch_) in ((sc2, bcc, ccl_c, cch_c),
                                          (sd2, bcd, cdl_c, cdh_c)):
                    eng.tensor_scalar(
                        out=oh[:, 0:512], in0=bc[:], scalar1=cl, scalar2=None,
                        op0=ALU.is_equal)
                    eng.tensor_scalar(
                        out=oh[:, 512:1024], in0=bc[:], scalar1=ch_, scalar2=None,
                        op0=ALU.is_equal)
                oh_of[i] = (sc2, sd2)

            def emit_h1(i):
                sc2, sd2 = oh_of.pop(i)
                hps = []
                for (mt, msl) in ((0, slice(0, P)), (1, slice(P, HID))):
                    mm = msl.stop - msl.start
                    hp = ps_h1.tile([mm, 512], f32, tag=f"h1_{mt}")
                    nc.tensor.matmul(out=hp[:], lhsT=a_k[0][:, msl],
                                     rhs=sc2[:, 0:512], start=True, stop=False)
                    nc.tensor.matmul(out=hp[:], lhsT=a_k[1][:, msl],
                                     rhs=sc2[0:HID - P, 512:1024],
                                     start=False, stop=False)
                    nc.tensor.matmul(out=hp[:], lhsT=bd_k[0][:, msl],
                                     rhs=sd2[:, 0:512], start=False, stop=False)
                    nc.tensor.matmul(out=hp[:], lhsT=bd_k[1][:, msl],
                                     rhs=sd2[:, 512:1024], start=False, stop=True)
                    hps.append(hp)
                h1ps_of[i] = hps

            def emit_h1relu(i):
                hps = h1ps_of.pop(i)
                h1s = []
                for mt, hp in enumerate(hps):
                    mm = P if mt == 0 else HID - P
                    hs = sbc.tile([mm, 512], bf16, tag=f"h1s_{mt}")
                    nc.vector.tensor_scalar_max(out=hs[:], in0=hp[:], scalar1=0.0)
                    h1s.append(hs)
                h1s_of[i] = h1s

            def emit_h2(i):
                h1s = h1s_of.pop(i)
                hps = []
                for (mt, msl) in ((0, slice(0, P)), (1, slice(P, HID))):
                    mm = msl.stop - msl.start
                    hp = ps_h2.tile([mm, 512], f32, tag=f"h2_{mt}")
                    nc.tensor.matmul(out=hp[:], lhsT=wf2a[:, msl], rhs=h1s[0][:],
                                     start=True, stop=False)
                    nc.tensor.matmul(out=hp[:], lhsT=wf2b[:, msl], rhs=h1s[1][:],
                                     start=False, stop=True)
                    hps.append(hp)
                h2ps_of[i] = hps

            def emit_h2relu(i):
                hps = h2ps_of.pop(i)
                nc.scalar.activation(out=h2s0_st[:], in_=hps[0][:], func=AF.Relu,
                                     bias=bf2a_c, scale=1.0)
                nc.scalar.activation(out=h2s1_st[0:HID - P, :], in_=hps[1][:],
                                     func=AF.Relu, bias=bf2b_c, scale=1.0)

            def emit_fm(i):
                fm = ps_fm.tile([8 + NDOSES, 512], f32, tag="fm")
                nc.tensor.matmul(out=fm[:], lhsT=fma, rhs=h2s0_st[:],
                                 start=True, stop=False)
                nc.tensor.matmul(out=fm[:], lhsT=fmb, rhs=h2s1_st[:],
                                 start=False, stop=True)
                fm_of[i] = fm

            def emit_softplus(i):
                fm = fm_of[i]
                spb = sbc.tile([8, 512], bf16, tag="spb")
                if USE_SOFTPLUS:
                    nc.scalar.activation(out=spb[:], in_=fm[0:8, :],
                                         func=AF.Softplus)
                else:
                    nc.scalar.activation(out=spb[:], in_=fm[0:8, :], func=AF.Exp)
                    nc.scalar.activation(out=spb[:], in_=spb[:], func=AF.Ln,
                                         bias=1.0, scale=1.0)
                spb_of[i] = spb

            def emit_l8(i):
                fm = fm_of[i]
                spb = spb_of.pop(i)
                nc.tensor.matmul(out=fm[:], lhsT=l8_sb, rhs=spb[:],
                                 start=False, stop=True, skip_group_check=True)

            def emit_mucopy(i):
                # rows 0:8 = spent f9 junk (not stored); rows 8:17 = mu
                g, pc = chunks[i]
                fm = fm_of.pop(i)
                n0 = g * GS + pc * 512
                muc = sbc.tile([8 + NDOSES, 512], f32, tag="muc")
                nc.vector.tensor_copy(out=muc[:], in_=fm[:])
                nc.sync.dma_start(out=mu9_s[:, n0:n0 + 512],
                                  in_=muc[8:8 + NDOSES, :])

            # prologue
            emit_qb(0)
            emit_bcopy(0)
            emit_onehot(0)

            mk_prs = lambda: ps_h1.tile([P, 512], f32, tag="h1_0", name="prs")
            for i in range(NCH):
                emit_h1(i)
                emit_h1relu(i)
                if i == NCH // 2 - 2:
                    # resolve piece 1 cs (its gather lands around now); the
                    # h1_0 bank's next chunk use is pc-1 (needs this anyway)
                    emit_resolve(1, mk_prs, names=("c",))
                if i + 1 < NCH and i != NCH // 2 - 1:
                    emit_qb(i + 1)
                    emit_bcopy(i + 1)
                    emit_onehot(i + 1)
                if i >= 1:
                    emit_h2(i - 1)
                    emit_h2relu(i - 1)
                    emit_fm(i - 1)
                    emit_softplus(i - 1)
                if i == NCH // 2 - 1:
                    # piece-1 dm resolve late in the iteration, then the
                    # deferred qb for the first pc-1 chunk
                    emit_resolve(1, mk_prs, names=("d",))
                    emit_qb(i + 1)
                    emit_bcopy(i + 1)
                    emit_onehot(i + 1)
                if i >= 2:
                    emit_l8(i - 2)
                    emit_mucopy(i - 2)
            for i in (NCH - 1,):
                emit_h2(i)
                emit_h2relu(i)
                emit_fm(i)
                emit_softplus(i)
            emit_l8(NCH - 2)
            emit_mucopy(NCH - 2)
            emit_l8(NCH - 1)
            emit_mucopy(NCH - 1)

    return _split_sync_waits(nc) if split_waits else nc


def _get_nc():
    if "nc" not in _NC_CACHE:
        _NC_CACHE["nc"] = build_nc()
    return _NC_CACHE["nc"]


def make_in_maps(inputs):
    idx = np.asarray(inputs["idx"], np.int64)
    tidx = np.asarray(inputs["tidx"], np.int64)
    cm = np.asarray(inputs["cell_map"]).astype(np.uint8)
    mi = np.asarray(inputs["is_missing"]).astype(np.uint8)
    dmv = np.asarray(inputs["drug_map"]).astype(np.uint8)
    cf = np.asarray(inputs["cell_features"], np.float32)
    me = np.asarray(inputs["missing_emb"], np.float32)
    de = np.asarray(inputs["drug_emb"], np.float32)
    W1 = np.asarray(inputs["W1"], np.float32)
    Wf1 = np.asarray(inputs["Wf1"], np.float32)
    Wf2 = np.asarray(inputs["Wf2"], np.float32)
    Wf3 = np.asarray(inputs["Wf3"], np.float32)
    b1 = np.asarray(inputs["b1"], np.float32)
    bf1 = np.asarray(inputs["bf1"], np.float32)
    bf2 = np.asarray(inputs["bf2"], np.float32)
    bf3 = np.asarray(inputs["bf3"], np.float32)

    # ---- bf16 blobs ----
    t1 = np.zeros((P, NT1), np_bf16)
    t2 = np.zeros((P, NT2), np_bf16)
    t3 = np.zeros((P, NT3), np_bf16)

    for kt in range(8):
        t1[:, T1_L["w1"][0] + kt * CEMB:T1_L["w1"][0] + (kt + 1) * CEMB] = \
            W1[kt * P:(kt + 1) * P, :].astype(np_bf16)
        t1[:, T1_L["cft"][0] + kt * 100:T1_L["cft"][0] + (kt + 1) * 100] = \
            cf[:100, kt * P:(kt + 1) * P].T.astype(np_bf16)
        t2[:, T2_L["wf1c"][0] + kt * HID:T2_L["wf1c"][0] + (kt + 1) * HID] = \
            Wf1[kt * P:(kt + 1) * P, :].astype(np_bf16)

    def put(blob, L, name, rows, arr):
        lo, hi = L[name]
        blob[rows[0]:rows[1], lo:hi] = arr.astype(np_bf16)

    put(t2, T2_L, "wf1d", (0, DEMB), Wf1[CEMB:, :])
    put(t2, T2_L, "deT", (0, DEMB), de.T)
    put(t2, T2_L, "meb", (0, 100), me)
    t2[:, T2_L["de"][0]:T2_L["de"][0] + DEMB] = de[0:P, :].astype(np_bf16)
    t2[:, T2_L["de"][0] + DEMB:T2_L["de"][0] + 2 * DEMB] = \
        de[P:NDRUG, :].astype(np_bf16)
    ts = np.zeros((P, NTS), np_bf16)
    put(ts, TS_L, "grp_bc", (0, NG),
        np.array([[1.0 if (p // 16) == g else 0.0 for p in range(P)]
                  for g in range(NG)], np.float32))
    put(ts, TS_L, "grp_rd", (0, P),
        np.array([[1.0 if (p // 16) == g else 0.0 for g in range(NG)]
                  for p in range(P)], np.float32))
    sel = np.zeros((NG, NG * P), np.float32)
    for g in range(NG):
        sel[g, g * P:(g + 1) * P] = 1.0
    put(ts, TS_L, "selg", (0, NG), sel)

    put(t3, T3_L, "wf2a", (0, P), Wf2[0:P, :])
    put(t3, T3_L, "wf2b", (0, HID - P), Wf2[P:HID, :])
    w3p = Wf3[:, [1, 2, 3, 4, 5, 6, 7, 8, 0]]
    b3p = bf3[[1, 2, 3, 4, 5, 6, 7, 8, 0]]
    fma = np.concatenate([w3p[0:P, 0:8], np.tile(w3p[0:P, 8:9], (1, 9))], axis=1)
    put(t3, T3_L, "fma", (0, P), fma)
    fmb = np.zeros((P, 17), np.float32)
    fmb[0:HID - P, 0:8] = w3p[P:HID, 0:8]
    fmb[0:HID - P, 8:17] = np.tile(w3p[P:HID, 8:9], (1, 9))
    fmb[P - 1, 0:8] = b3p[0:8]
    fmb[P - 1, 8:17] = b3p[8]
    put(t3, T3_L, "fmb", (0, P), fmb)
    l8 = np.zeros((8, 17), np.float32)
    l8[:, 8:17] = np.triu(np.ones((8, NDOSES), np.float32), 1)
    put(t3, T3_L, "l8", (0, 8), l8)
    put(t3, T3_L, "ones512", (0, 1), np.ones((1, 512), np.float32))

    br = np.zeros((1, NBR), np.float32)
    br[0, BR_L["b1r"][0]:BR_L["b1r"][1]] = b1
    br[0, BR_L["bf1r"][0]:BR_L["bf1r"][1]] = bf1
    br[0, BR_L["onesr"][0]:BR_L["onesr"][1]] = 1.0

    bc = np.zeros((P, NBC), np.float32)
    pp = np.arange(P)
    bc[:, BC_L["qi"][0]] = pp % 16
    bc[:, BC_L["ccl"][0]] = np.where(pp < 100, pp, pp + 28)
    bc[:, BC_L["cch"][0]] = np.where(pp < HID - P, pp + 156, 1000)
    bc[:, BC_L["cdl"][0]] = pp
    bc[:, BC_L["cdh"][0]] = pp + P
    bc[:, BC_L["bf2a"][0]] = bf2[0:P]
    bc[0:HID - P, BC_L["bf2b"][0]] = bf2[P:HID]

    shared = dict(
        blob_t1=np.ascontiguousarray(t1),
        blob_t2=np.ascontiguousarray(t2),
        blob_t3=np.ascontiguousarray(t3),
        blob_ts=np.ascontiguousarray(ts),
        brow=np.ascontiguousarray(br),
        bcol=np.ascontiguousarray(bc),
        # slab layout replicated to all 8 groups on the host (layout-only op)
        csmi=np.ascontiguousarray(
            np.tile((cm | (mi << 7)).reshape(16, SLAB), (NG, 1))),
        dm_map=np.ascontiguousarray(np.tile(dmv.reshape(16, SLAB), (NG, 1))),
    )

    def wrap16(vals):
        # vals [8192] in sample order k (g = k>>10, j = k&1023)
        # -> [128, 64] at [16g + (j & 15), j >> 4]
        v = vals.reshape(NG, GS // 16, 16)        # [g, j_hi, j_lo]
        v = np.transpose(v, (0, 2, 1))            # [g, j_lo, j_hi]
        return np.ascontiguousarray(v.reshape(P, GS // 16))

    in_maps = []
    for c in range(NCORES):
        ic = idx[c * BS:(c + 1) * BS]
        tc_ = tidx[c * BS:(c + 1) * BS]
        m = dict(shared)
        m["u_idx"] = wrap16((ic & (SLAB - 1)).astype(np.uint16))
        m["u_tidx"] = wrap16((tc_ & (SLAB - 1)).astype(np.uint16))
        m["q_idx"] = np.ascontiguousarray(
            (ic >> 14).astype(np_bf16).reshape(NG, GS))
        m["q_tidx"] = np.ascontiguousarray(
            (tc_ >> 14).astype(np_bf16).reshape(NG, GS))
        in_maps.append(m)
    return in_maps


def kernel(**inputs):
    nc = _get_nc()
    in_maps = make_in_maps(inputs)
    last_err = None
    for _attempt in range(3):
        try:
            res = run_bass_kernel_spmd(nc, in_maps, core_ids=list(range(NCORES)))
            return np.concatenate(
                [np.ascontiguousarray(res.results[c]["mu9_s"].T)
                 for c in range(NCORES)], axis=0)
        except Exception as e:  # wedged device sometimes recovers on retry
            last_err = e
    raise last_err


# revision 74
# speedup vs baseline: 1.0251x; 1.0251x over previous
"""Trainium2 Bass kernel for nn_DrugResponsePrior (embedding_lookup).

Spec guarantees: cell_map < 100, is_missing in {0,1}, drug_map < 256.  Each
row's result depends only on the cell state cs = cell_map[idx] +
100*is_missing[idx] (200 states) and dm = drug_map[tidx] (256 drugs).

Fully data-parallel (8 cores x 8192 samples, no collectives).  Per core:
  1. Host bit-packs csmi = cell_map | (is_missing << 7) (u8 - a pure bit
     repack; the state CODE cm + 128*mi is exact in bf16).  csmi/drug_map are
     loaded in a 16-slab SBUF layout (partition 16g+r holds entries
     [r*16384, (r+1)*16384) for every group g).
  2. Two GPSIMD indirect_copy gathers per 4096-sample piece fetch the 16
     slab candidates per sample; a one-hot mask over idx>>14 (grp_bc matmul +
     is_eq) and a group-reduce matmul produce v8 = per-sample code [8, 1024].
  3. Tables built once on device: A = l2n(cell emb) @ Wf1c + bf1 ([200,200])
     and Bd = l2n(drug_emb) @ Wf1d ([256,200]), bf16.
  4. Per 512-sample chunk: gpsimd partition_broadcast expands the codes to
     [128, 512]; two DVE is_eq (4x mode, bf16) build one-hot matrices; bf16
     matmuls run the MLP: h1 = relu(A^T Sc + Bd^T Sd), h2 = relu(Wf2^T h1 +
     bf2), fm = [fwd(1:9); mu-base] via one packed [.,17] lhsT, softplus on
     scalar engine, one L8 matmul accumulates the cumsum into the mu rows.
  Chunks are software-pipelined depth 3 so every engine streams without
  gaps (keeps the PE p-state at full clock).

All params ride in two packed blob tensors ([128, N] with large contiguous
partition lines) so the whole param load is 2 DMAs - the baseline's ~16k
small DMA descriptors were the main bottleneck.

All reference math runs on device; the host only reshapes/transposes/casts
inputs, bit-packs the two sub-byte index tables, and slices idx/tidx (pure
index arithmetic: & 16383, >> 14).
"""
import os
import sys

if "/opt/trn_rl_repo" not in sys.path:
    sys.path.insert(0, "/opt/trn_rl_repo")

# Neither CoreSim nor this neuronxcc's activation tables implement Softplus;
# softplus is computed as ln(1 + exp(x)) in two scalar passes.
USE_SOFTPLUS = False

import numpy as np
import ml_dtypes

import concourse.bass as bass
import concourse.bass_isa as bass_isa
import concourse.mybir as mybir
import concourse.tile as tile
from concourse.bass_utils import run_bass_kernel_spmd

f32 = mybir.dt.float32
bf16 = mybir.dt.bfloat16
u16 = mybir.dt.uint16
u8 = mybir.dt.uint8
np_bf16 = ml_dtypes.bfloat16

B = 65536
R = 262144
NDRUG = 256
NFEAT = 1024
CEMB = 1024
DEMB = 128
HID = 200
NDOSES = 9
NCORES = 8

BS = B // NCORES            # 8192 samples per core
P = 128
NG = 8                      # gpsimd groups (16 partitions each)
GS = BS // NG               # 1024 samples per group
SLAB = R // 16              # 16384 entries per slab partition
EPS = 1e-12

_NC_CACHE = {}

# ---------------- packed blob layouts (host & device share these) ----------
def _layout(specs):
    out, off = {}, 0
    for name, n in specs:
        out[name] = (off, off + n)
        off += n
    return out, off

# bf16 param blobs, split by when the device needs them (3 staged DMAs)
T1_L, NT1 = _layout([
    ("w1", 8 * CEMB),        # 8 k-tiles [128, 1024]
    ("cft", 8 * 100),        # 8 k-tiles [128, 100]
])
T2_L, NT2 = _layout([
    ("wf1c", 8 * HID),       # 8 k-tiles [128, 200]
    ("wf1d", HID),           # [128, 200]
    ("deT", NDRUG),          # [128, 256] drug_emb^T
    ("meb", CEMB),           # rows 0:100 = missing_emb
    ("de", 2 * DEMB),        # drug_emb [256, 128] as two [128, 128] tiles
])
# tiny early-loaded bf16 constants (lookup resolve must not wait for t2)
TS_L, NTS = _layout([
    ("grp_bc", P),           # rows 0:8: [g, p] = (p//16 == g)
    ("grp_rd", NG),          # [p, g] = (p//16 == g)
    ("selg", NG * P),        # block g: [g', p] = (g' == g)  (bcast lhsT)
])
T3_L, NT3 = _layout([
    ("wf2a", HID),           # Wf2[0:128, :]
    ("wf2b", HID),           # rows 0:72 = Wf2[128:200, :]
    ("fma", 17),             # [Wf3p[0:128, 0:8] | tile(col base, 9)]
    ("fmb", 17),             # rows 0:72 = Wf3p[128:200]; row 127 = biases
    ("l8", 17),              # rows 0:8: cols 0:8 zero, cols 8:17 (k < o)
    ("ones512", 512),        # row 0 = 1.0 (DMA'd to h2s1_st row 127)
])
# single-row f32 tensor (bias rows for matmul rhs; base partition 0)
BR_L, NBR = _layout([
    ("b1r", CEMB),
    ("bf1r", HID),
    ("onesr", P),
])
# per-partition f32 columns [128, NBC]
BC_L, NBC = _layout([
    ("qi", 1),               # p % 16
    ("ccl", 1),              # cs code, one-hot block lo
    ("cch", 1),              # cs code, block hi
    ("cdl", 1),              # dm code lo
    ("cdh", 1),              # dm code hi
    ("bf2a", 1),
    ("bf2b", 1),
])


def _split_sync_waits(nc, limit=1):
    """The walrus accepts at most one sync-wait per instruction; hoist excess
    waits onto same-engine NoOps inserted just before."""
    ctr = 0
    for bb in nc.main_func.blocks:
        new_list = []
        for inst in bb.instructions:
            si = inst.sync_info
            if si is not None and si.on_wait and len(si.on_wait) > limit:
                waits = list(si.on_wait)
                head, tail = waits[:-limit], waits[-limit:]
                for j in range(0, len(head), limit):
                    nop = mybir.InstNoOp(name=f"waitnop-{ctr}", engine=inst.engine)
                    ctr += 1
                    nop.sync_info = mybir.SyncInfo(
                        on_wait=list(head[j : j + limit]), on_update=[]
                    )
                    new_list.append(nop)
                inst.sync_info = mybir.SyncInfo(
                    on_wait=list(tail),
                    on_update=list(si.on_update) if si.on_update else [],
                )
            new_list.append(inst)
        bb.instructions[:] = new_list
    return nc


def build_nc(split_waits=True):
    nc = bass.Bass(num_devices=NCORES)
    AF = mybir.ActivationFunctionType
    ALU = mybir.AluOpType

    # ---------------- kernel I/O ----------------
    blob_t1 = nc.dram_tensor("blob_t1", [P, NT1], bf16, kind="ExternalInput")
    blob_t2 = nc.dram_tensor("blob_t2", [P, NT2], bf16, kind="ExternalInput")
    blob_t3 = nc.dram_tensor("blob_t3", [P, NT3], bf16, kind="ExternalInput")
    blob_ts = nc.dram_tensor("blob_ts", [P, NTS], bf16, kind="ExternalInput")
    brow = nc.dram_tensor("brow", [1, NBR], f32, kind="ExternalInput")
    bcol = nc.dram_tensor("bcol", [P, NBC], f32, kind="ExternalInput")
    # host pre-replicates the 16-slab tables to all 8 groups (contiguous
    # DMA: ~266 GB/s vs ~80 GB/s for a stride-0 replicating read)
    csmi = nc.dram_tensor("csmi", [P, SLAB], u8, kind="ExternalInput")
    dm_map = nc.dram_tensor("dm_map", [P, SLAB], u8, kind="ExternalInput")
    u_idx = nc.dram_tensor("u_idx", [P, GS // 16], u16, kind="ExternalInput")
    u_tidx = nc.dram_tensor("u_tidx", [P, GS // 16], u16, kind="ExternalInput")
    q_idx = nc.dram_tensor("q_idx", [NG, GS], bf16, kind="ExternalInput")
    q_tidx = nc.dram_tensor("q_tidx", [NG, GS], bf16, kind="ExternalInput")
    mu9_s = nc.dram_tensor("mu9_s", [NDOSES, BS], f32, kind="ExternalOutput")

    with tile.TileContext(nc) as tc, \
            tc.tile_pool(name="sbw", bufs=1) as sbw, \
            tc.tile_pool(name="sb", bufs=1) as sb:

        # ---- setup DMAs ----
        # queue A (sync -> DMA engines 0-7): index tensors + slabs
        # queue B (scalar -> DMA engines 8-15): param blobs, staged
        u_idx_sb = sb.tile([P, GS // 16], u16)
        u_tidx_sb = sb.tile([P, GS // 16], u16)
        q_idx_sb = sb.tile([NG, GS], bf16)
        q_tidx_sb = sb.tile([NG, GS], bf16)
        bc_sb = sb.tile([P, NBC], f32)
        nc.sync.dma_start(out=bc_sb[:], in_=bcol[:])
        nc.sync.dma_start(out=u_idx_sb[:], in_=u_idx[:])
        nc.sync.dma_start(out=u_tidx_sb[:], in_=u_tidx[:])
        nc.sync.dma_start(out=q_idx_sb[:], in_=q_idx[:])
        nc.sync.dma_start(out=q_tidx_sb[:], in_=q_tidx[:])
        cs_slab = sbw.tile([P, SLAB], u8)
        dm_slab = sbw.tile([P, SLAB], u8)
        t1_sb = sbw.tile([P, NT1], bf16)
        # A queue: cs table first (gathers gate everything downstream)
        nc.sync.dma_start(out=cs_slab[:], in_=csmi[:])
        # B queue: tiny consts, then dm table
        br_sb = sb.tile([1, NBR], f32)
        nc.scalar.dma_start(out=br_sb[:], in_=brow[:])
        ts_sb = sb.tile([P, NTS], bf16)
        nc.scalar.dma_start(out=ts_sb[:], in_=blob_ts[:])
        nc.scalar.dma_start(out=dm_slab[:], in_=dm_map[:])
        # t1 streamed per k-tile, alternating queues, so the P100 matmuls
        # start as soon as each tile lands instead of after the whole blob
        for kt in range(8):
            eng = nc.sync if kt % 2 == 0 else nc.scalar
            w0, _ = T1_L["w1"]
            c0, _ = T1_L["cft"]
            eng.dma_start(out=t1_sb[:, w0 + kt * CEMB:w0 + (kt + 1) * CEMB],
                          in_=blob_t1[:, w0 + kt * CEMB:w0 + (kt + 1) * CEMB])
            eng.dma_start(out=t1_sb[:, c0 + kt * 100:c0 + (kt + 1) * 100],
                          in_=blob_t1[:, c0 + kt * 100:c0 + (kt + 1) * 100])
        t2_sb = sbw.tile([P, NT2], bf16)
        nc.scalar.dma_start(out=t2_sb[:], in_=blob_t2[:])
        t3_sb = sbw.tile([P, NT3], bf16)
        nc.scalar.dma_start(out=t3_sb[:], in_=blob_t3[:])

        # blob views
        me_sb = t2_sb[0:100, T2_L["meb"][0]:T2_L["meb"][1]]
        b1_row = br_sb[:, BR_L["b1r"][0]:BR_L["b1r"][1]]
        bf1_row = br_sb[:, BR_L["bf1r"][0]:BR_L["bf1r"][1]]
        ones100 = br_sb[:, BR_L["onesr"][0]:BR_L["onesr"][0] + 100]
        ones128 = br_sb[:, BR_L["onesr"][0]:BR_L["onesr"][1]]
        de0 = t2_sb[:, T2_L["de"][0]:T2_L["de"][0] + DEMB]
        de1 = t2_sb[:, T2_L["de"][0] + DEMB:T2_L["de"][0] + 2 * DEMB]
        qi_c = bc_sb[:, BC_L["qi"][0]:BC_L["qi"][1]]
        ccl_c = bc_sb[:, BC_L["ccl"][0]:BC_L["ccl"][1]]
        cch_c = bc_sb[:, BC_L["cch"][0]:BC_L["cch"][1]]
        cdl_c = bc_sb[:, BC_L["cdl"][0]:BC_L["cdl"][1]]
        cdh_c = bc_sb[:, BC_L["cdh"][0]:BC_L["cdh"][1]]
        bf2a_c = bc_sb[:, BC_L["bf2a"][0]:BC_L["bf2a"][1]]
        bf2b_c = bc_sb[0:72, BC_L["bf2b"][0]:BC_L["bf2b"][1]]
        w1_kt = [t1_sb[:, T1_L["w1"][0] + k * CEMB:T1_L["w1"][0] + (k + 1) * CEMB]
                 for k in range(8)]
        cft_kt = [t1_sb[:, T1_L["cft"][0] + k * 100:T1_L["cft"][0] + (k + 1) * 100]
                  for k in range(8)]
        wf1c_kt = [t2_sb[:, T2_L["wf1c"][0] + k * HID:T2_L["wf1c"][0] + (k + 1) * HID]
                   for k in range(8)]
        wf1d_sb = t2_sb[:, T2_L["wf1d"][0]:T2_L["wf1d"][1]]
        deT_sb = t2_sb[:, T2_L["deT"][0]:T2_L["deT"][1]]
        grp_bc = ts_sb[0:NG, TS_L["grp_bc"][0]:TS_L["grp_bc"][1]]
        grp_rd = ts_sb[:, TS_L["grp_rd"][0]:TS_L["grp_rd"][1]]
        selg = [ts_sb[0:NG, TS_L["selg"][0] + g * P:TS_L["selg"][0] + (g + 1) * P]
                for g in range(NG)]
        wf2a = t3_sb[:, T3_L["wf2a"][0]:T3_L["wf2a"][1]]
        wf2b = t3_sb[0:72, T3_L["wf2b"][0]:T3_L["wf2b"][1]]
        fma = t3_sb[:, T3_L["fma"][0]:T3_L["fma"][1]]
        fmb = t3_sb[:, T3_L["fmb"][0]:T3_L["fmb"][1]]
        l8_sb = t3_sb[0:8, T3_L["l8"][0]:T3_L["l8"][1]]

        # static h2 tiles (relu outputs; h2s1 rows 72:127 zero, row 127 ones
        # so fmb's row 127 supplies the biases)
        h2s0_st = sbw.tile([P, 512], bf16)
        h2s1_st = sbw.tile([P, 512], bf16)
        nc.vector.memset(h2s1_st[:], 0.0)
        nc.sync.dma_start(
            out=h2s1_st[P - 1:P, :],
            in_=blob_t3[0:1, T3_L["ones512"][0]:T3_L["ones512"][1]])

        # lookup state (lives across the table scope and the chunk scope)
        g_cs = sb.tile([P, GS], u8)
        g_dm = sb.tile([P, GS], u8)
        v8_cs = sb.tile([NG, GS], bf16)
        v8_dm = sb.tile([NG, GS], bf16)

        def emit_resolve(t, mk_ps, names=("c", "d")):
            jsl = slice(t * 512, (t + 1) * 512)
            for (gt, qt, v8t, nm) in ((g_cs, q_idx_sb, v8_cs, "c"),
                                      (g_dm, q_tidx_sb, v8_dm, "d")):
                if nm not in names:
                    continue
                qb = mk_ps()
                nc.tensor.matmul(out=qb[:], lhsT=grp_bc, rhs=qt[:, jsl],
                                 start=True, stop=True)
                qmask = sb.tile([P, 512], bf16, tag=f"qmask_{nm}",
                                name=f"qmask_{nm}")
                nc.vector.tensor_scalar(
                    out=qmask[:], in0=qb[:], scalar1=qi_c, scalar2=None,
                    op0=ALU.is_equal)
                gf = sb.tile([P, 512], bf16, tag=f"gf_{nm}", name=f"gf_{nm}")
                nc.vector.tensor_copy(out=gf[:], in_=gt[:, jsl])
                nc.vector.tensor_tensor(out=gf[:], in0=gf[:], in1=qmask[:],
                                        op=ALU.mult)
                vpf = mk_ps()
                nc.tensor.matmul(out=vpf[0:NG, :], lhsT=grp_rd, rhs=gf[:],
                                 start=True, stop=True)
                nc.vector.tensor_copy(out=v8t[:, jsl], in_=vpf[0:NG, :])

        # ======== table construction: A [200,200], Bd [256,200] (bf16) ======
        a_k = []
        bd_k = []
        with (
            tc.tile_pool(name="ps_tb", bufs=1, space="PSUM") as ps_tb,
            tc.tile_pool(name="ps_tr", bufs=3, space="PSUM") as ps_tr,
            tc.tile_pool(name="sbt", bufs=1) as sbt,
        ):
            from concourse.masks import make_identity
            ident = sbt.tile([P, P], bf16)
            make_identity(nc, ident[:])

            # P100 = relu(cf @ W1 + b1)  [100, 1024]; kt-major so each matmul
            # runs as soon as its streamed k-tile DMA lands
            p_sb = sbt.tile([100, CEMB], bf16)
            pps = [ps_tb.tile([100, 512], f32, tag=f"pshard{nh}",
                              name=f"pps{nh}") for nh in range(2)]
            for kt in range(8):
                for nh in range(2):
                    nc.tensor.matmul(
                        out=pps[nh][:], lhsT=cft_kt[kt],
                        rhs=w1_kt[kt][:, nh * 512:(nh + 1) * 512],
                        start=(kt == 0), stop=False)
            for nh in range(2):
                nc.tensor.matmul(
                    out=pps[nh][:], lhsT=ones100,
                    rhs=b1_row[:, nh * 512:(nh + 1) * 512], start=False, stop=True)
                nc.scalar.activation(
                    out=p_sb[:, nh * 512:(nh + 1) * 512], in_=pps[nh][:],
                    func=AF.Relu)

            # l2 norm scales for present / missing rows
            sq = sbt.tile([100, CEMB], f32)
            ssp = sbt.tile([100, 1], f32)
            ssm = sbt.tile([100, 1], f32)
            nc.scalar.activation(out=sq[:], in_=p_sb[:], func=AF.Square)
            nc.vector.reduce_sum(out=ssp[:], in_=sq[:], axis=mybir.AxisListType.X)
            nc.scalar.activation(out=sq[:], in_=me_sb, func=AF.Square)
            nc.vector.reduce_sum(out=ssm[:], in_=sq[:], axis=mybir.AxisListType.X)
            for ss in (ssp, ssm):
                nc.scalar.activation(out=ss[:], in_=ss[:], func=AF.Sqrt)
                nc.vector.tensor_scalar_max(out=ss[:], in0=ss[:], scalar1=EPS)
                nc.vector.reciprocal(out=ss[:], in_=ss[:])
            nc.vector.tensor_scalar_mul(out=p_sb[:], in0=p_sb[:], scalar1=ssp[:])
            nc.vector.tensor_scalar_mul(out=me_sb, in0=me_sb, scalar1=ssm[:])

            # CnT k-tiles [128, 200] bf16 (cols: 100 present + 100 missing)
            cnt_kt = []
            for kt in range(8):
                t = sbt.tile([P, 2 * 100], bf16, tag=f"cnt_{kt}")
                for (src, co) in ((p_sb[:], 0), (me_sb, 100)):
                    tp = ps_tr.tile([P, 100], bf16, tag="tr")
                    nc.tensor.transpose(
                        out=tp[:], in_=src[:, kt * P:(kt + 1) * P],
                        identity=ident[:100, :100])
                    nc.vector.tensor_copy(out=t[:, co:co + 100], in_=tp[:])
                cnt_kt.append(t)

            # A tiles (states on partitions): a_k[0] [128, 200], a_k[1] [72, 200]
            for (mt, msl) in ((0, slice(0, P)), (1, slice(P, HID))):
                mm = msl.stop - msl.start
                aps = ps_tb.tile([P, HID], f32, tag="a")
                for kt in range(8):
                    nc.tensor.matmul(
                        out=aps[:mm, :], lhsT=cnt_kt[kt][:, msl],
                        rhs=wf1c_kt[kt], start=(kt == 0), stop=False)
                nc.tensor.matmul(
                    out=aps[:mm, :], lhsT=ones128[:, :mm], rhs=bf1_row,
                    start=False, stop=True)
                t = sb.tile([mm, HID], bf16, tag=f"a_{mt}")
                nc.vector.tensor_copy(out=t[:], in_=aps[:mm, :])
                a_k.append(t)

            # drug tiles: per-drug l2 recip + Bd [128, 200] bf16 x2
            for (mt, de_p) in ((0, de0), (1, de1)):
                sqd = sbt.tile([P, DEMB], f32, tag="sqd")
                rd = sbt.tile([P, 1], f32, tag=f"rd_{mt}")
                nc.scalar.activation(out=sqd[:], in_=de_p, func=AF.Square)
                nc.vector.reduce_sum(out=rd[:], in_=sqd[:], axis=mybir.AxisListType.X)
                nc.scalar.activation(out=rd[:], in_=rd[:], func=AF.Sqrt)
                nc.vector.tensor_scalar_max(out=rd[:], in0=rd[:], scalar1=EPS)
                nc.vector.reciprocal(out=rd[:], in_=rd[:])
                bps = ps_tb.tile([P, HID], f32, tag="a")
                nc.tensor.matmul(out=bps[:], lhsT=deT_sb[:, mt * P:(mt + 1) * P],
                                 rhs=wf1d_sb, start=True, stop=True)
                t = sb.tile([P, HID], bf16, tag=f"bd_{mt}")
                nc.vector.tensor_scalar_mul(out=t[:], in0=bps[:], scalar1=rd[:])
                bd_k.append(t)

            # ======== lookup gathers (gpsimd runs these back to back) ========
            for t in range(2):
                nc.gpsimd.indirect_copy(
                    out=g_cs[:, t * 512:(t + 1) * 512].rearrange(
                        "p (n one) -> p n one", one=1),
                    data=cs_slab[:], idxs=u_idx_sb[:, t * 32:(t + 1) * 32],
                    i_know_ap_gather_is_preferred=True)
                nc.gpsimd.indirect_copy(
                    out=g_dm[:, t * 512:(t + 1) * 512].rearrange(
                        "p (n one) -> p n one", one=1),
                    data=dm_slab[:], idxs=u_tidx_sb[:, t * 32:(t + 1) * 32],
                    i_know_ap_gather_is_preferred=True)
            # piece 0 resolved here; piece 1 resolved mid-chunk-stream so the
            # engine FIFOs don't head-of-line block on its gathers
            emit_resolve(0, lambda: ps_tb.tile([P, 512], f32, tag="pqb",
                                               name="pqb"))

        # ======== per-chunk pipeline ========
        chunks = [(g, pc) for pc in range(2) for g in range(NG)]
        NCH = len(chunks)

        with (
            tc.tile_pool(name="ps_h1", bufs=1, space="PSUM") as ps_h1,
            tc.tile_pool(name="ps_h2", bufs=1, space="PSUM") as ps_h2,
            tc.tile_pool(name="ps_fm", bufs=2, space="PSUM") as ps_fm,
            tc.tile_pool(name="ps_qb", bufs=1, space="PSUM") as ps_qb,
            tc.tile_pool(name="sbc", bufs=2) as sbc,
        ):
            bc_of, oh_of, h1ps_of, h1s_of, h2ps_of, fm_of, spb_of = \
                {}, {}, {}, {}, {}, {}, {}

            def emit_qb(i):
                # broadcast codes of chunk i's group to all 128 partitions
                g, pc = chunks[i]
                jsl = slice(pc * 512, (pc + 1) * 512)
                qbc = ps_qb.tile([P, 512], f32, tag="qbc")
                qbd = ps_qb.tile([P, 512], f32, tag="qbd")
                nc.tensor.matmul(out=qbc[:], lhsT=selg[g], rhs=v8_cs[:, jsl],
                                 start=True, stop=True)
                nc.tensor.matmul(out=qbd[:], lhsT=selg[g], rhs=v8_dm[:, jsl],
                                 start=True, stop=True)
                bc_of[i] = (qbc, qbd)

            def emit_bcopy(i):
                # bf16 SBUF copies so the is_eq runs in the DVE 4x mode;
                # both on scalar (DVE is the tighter engine)
                qbc, qbd = bc_of.pop(i)
                bcc = sbc.tile([P, 512], bf16, tag="bcc")
                bcd = sbc.tile([P, 512], bf16, tag="bcd")
                nc.scalar.activation(out=bcc[:], in_=qbc[:], func=AF.Copy)
                nc.vector.tensor_copy(out=bcd[:], in_=qbd[:])
                bc_of[i] = (bcc, bcd)

            def emit_onehot(i):
                eng = nc.vector
                bcc, bcd = bc_of.pop(i)
                sc2 = sbc.tile([P, 1024], bf16, tag="sc2")
                sd2 = sbc.tile([P, 1024], bf16, tag="sd2")
                for (oh, bc, cl, ch_) in ((sc2, bcc, ccl_c, cch_c),
                                          (sd2, bcd, cdl_c, cdh_c)):
                    eng.tensor_scalar(
                        out=oh[:, 0:512], in0=bc[:], scalar1=cl, scalar2=None,
                        op0=ALU.is_equal)
                    eng.tensor_scalar(
                        out=oh[:, 512:1024], in0=bc[:], scalar1=ch_, scalar2=None,
                        op0=ALU.is_equal)
                oh_of[i] = (sc2, sd2)

            def emit_h1(i):
                sc2, sd2 = oh_of.pop(i)
                hps = []
                for (mt, msl) in ((0, slice(0, P)), (1, slice(P, HID))):
                    mm = msl.stop - msl.start
                    hp = ps_h1.tile([mm, 512], f32, tag=f"h1_{mt}")
                    nc.tensor.matmul(out=hp[:], lhsT=a_k[0][:, msl],
                                     rhs=sc2[:, 0:512], start=True, stop=False)
                    nc.tensor.matmul(out=hp[:], lhsT=a_k[1][:, msl],
                                     rhs=sc2[0:HID - P, 512:1024],
                                     start=False, stop=False)
                    nc.tensor.matmul(out=hp[:], lhsT=bd_k[0][:, msl],
                                     rhs=sd2[:, 0:512], start=False, stop=False)
                    nc.tensor.matmul(out=hp[:], lhsT=bd_k[1][:, msl],
                                     rhs=sd2[:, 512:1024], start=False, stop=True)
                    hps.append(hp)
                h1ps_of[i] = hps

            def emit_h1relu(i):
                hps = h1ps_of.pop(i)
                h1s = []
                for mt, hp in enumerate(hps):
                    mm = P if mt == 0 else HID - P
                    hs = sbc.tile([mm, 512], bf16, tag=f"h1s_{mt}")
                    nc.vector.tensor_scalar_max(out=hs[:], in0=hp[:], scalar1=0.0)
                    h1s.append(hs)
                h1s_of[i] = h1s

            def emit_h2(i):
                h1s = h1s_of.pop(i)
                hps = []
                for (mt, msl) in ((0, slice(0, P)), (1, slice(P, HID))):
                    mm = msl.stop - msl.start
                    hp = ps_h2.tile([mm, 512], f32, tag=f"h2_{mt}")
                    nc.tensor.matmul(out=hp[:], lhsT=wf2a[:, msl], rhs=h1s[0][:],
                                     start=True, stop=False)
                    nc.tensor.matmul(out=hp[:], lhsT=wf2b[:, msl], rhs=h1s[1][:],
                                     start=False, stop=True)
                    hps.append(hp)
                h2ps_of[i] = hps

            def emit_h2relu(i):
                hps = h2ps_of.pop(i)
                nc.scalar.activation(out=h2s0_st[:], in_=hps[0][:], func=AF.Relu,
                                     bias=bf2a_c, scale=1.0)
                nc.scalar.activation(out=h2s1_st[0:HID - P, :], in_=hps[1][:],
                                     func=AF.Relu, bias=bf2b_c, scale=1.0)

            def emit_fm(i):
                fm = ps_fm.tile([8 + NDOSES, 512], f32, tag="fm")
                nc.tensor.matmul(out=fm[:], lhsT=fma, rhs=h2s0_st[:],
                                 start=True, stop=False)
                nc.tensor.matmul(out=fm[:], lhsT=fmb, rhs=h2s1_st[:],
                                 start=False, stop=True)
                fm_of[i] = fm

            def emit_softplus(i):
                fm = fm_of[i]
                spb = sbc.tile([8, 512], bf16, tag="spb")
                if USE_SOFTPLUS:
                    nc.scalar.activation(out=spb[:], in_=fm[0:8, :],
                                         func=AF.Softplus)
                else:
                    nc.scalar.activation(out=spb[:], in_=fm[0:8, :], func=AF.Exp)
                    nc.scalar.activation(out=spb[:], in_=spb[:], func=AF.Ln,
                                         bias=1.0, scale=1.0)
                spb_of[i] = spb

            def emit_l8(i):
                fm = fm_of[i]
                spb = spb_of.pop(i)
                nc.tensor.matmul(out=fm[:], lhsT=l8_sb, rhs=spb[:],
                                 start=False, stop=True, skip_group_check=True)

            def emit_mucopy(i):
                # rows 0:8 = spent f9 junk (not stored); rows 8:17 = mu
                g, pc = chunks[i]
                fm = fm_of.pop(i)
                n0 = g * GS + pc * 512
                muc = sbc.tile([8 + NDOSES, 512], f32, tag="muc")
                nc.vector.tensor_copy(out=muc[:], in_=fm[:])
                nc.sync.dma_start(out=mu9_s[:, n0:n0 + 512],
                                  in_=muc[8:8 + NDOSES, :])

            # prologue
            emit_qb(0)
            emit_bcopy(0)
            emit_onehot(0)

            mk_prs = lambda: ps_h1.tile([P, 512], f32, tag="h1_0", name="prs")
            for i in range(NCH):
                emit_h1(i)
                emit_h1relu(i)
                if i == NCH // 2 - 2:
                    # resolve piece 1 cs (its gather lands around now); the
                    # h1_0 bank's next chunk use is pc-1 (needs this anyway)
                    emit_resolve(1, mk_prs, names=("c",))
                if i + 1 < NCH and i != NCH // 2 - 1:
                    emit_qb(i + 1)
                    emit_bcopy(i + 1)
                    emit_onehot(i + 1)
                if i >= 1:
                    emit_h2(i - 1)
                    emit_h2relu(i - 1)
                    emit_fm(i - 1)
                    emit_softplus(i - 1)
                if i == NCH // 2 - 1:
                    # piece-1 dm resolve late in the iteration, then the
                    # deferred qb for the first pc-1 chunk
                    emit_resolve(1, mk_prs, names=("d",))
                    emit_qb(i + 1)
                    emit_bcopy(i + 1)
                    emit_onehot(i + 1)
                if i >= 2:
                    emit_l8(i - 2)
                    emit_mucopy(i - 2)
            for i in (NCH - 1,):
                emit_h2(i)
                emit_h2relu(i)
                emit_fm(i)
                emit_softplus(i)
            emit_l8(NCH - 2)
            emit_mucopy(NCH - 2)
            emit_l8(NCH - 1)
            emit_mucopy(NCH - 1)

    return _split_sync_waits(nc) if split_waits else nc


def _get_nc():
    if "nc" not in _NC_CACHE:
        _NC_CACHE["nc"] = build_nc()
    return _NC_CACHE["nc"]


def make_in_maps(inputs):
    idx = np.asarray(inputs["idx"], np.int64)
    tidx = np.asarray(inputs["tidx"], np.int64)
    cm = np.asarray(inputs["cell_map"]).astype(np.uint8)
    mi = np.asarray(inputs["is_missing"]).astype(np.uint8)
    dmv = np.asarray(inputs["drug_map"]).astype(np.uint8)
    cf = np.asarray(inputs["cell_features"], np.float32)
    me = np.asarray(inputs["missing_emb"], np.float32)
    de = np.asarray(inputs["drug_emb"], np.float32)
    W1 = np.asarray(inputs["W1"], np.float32)
    Wf1 = np.asarray(inputs["Wf1"], np.float32)
    Wf2 = np.asarray(inputs["Wf2"], np.float32)
    Wf3 = np.asarray(inputs["Wf3"], np.float32)
    b1 = np.asarray(inputs["b1"], np.float32)
    bf1 = np.asarray(inputs["bf1"], np.float32)
    bf2 = np.asarray(inputs["bf2"], np.float32)
    bf3 = np.asarray(inputs["bf3"], np.float32)

    # ---- bf16 blobs ----
    t1 = np.zeros((P, NT1), np_bf16)
    t2 = np.zeros((P, NT2), np_bf16)
    t3 = np.zeros((P, NT3), np_bf16)

    for kt in range(8):
        t1[:, T1_L["w1"][0] + kt * CEMB:T1_L["w1"][0] + (kt + 1) * CEMB] = \
            W1[kt * P:(kt + 1) * P, :].astype(np_bf16)
        t1[:, T1_L["cft"][0] + kt * 100:T1_L["cft"][0] + (kt + 1) * 100] = \
            cf[:100, kt * P:(kt + 1) * P].T.astype(np_bf16)
        t2[:, T2_L["wf1c"][0] + kt * HID:T2_L["wf1c"][0] + (kt + 1) * HID] = \
            Wf1[kt * P:(kt + 1) * P, :].astype(np_bf16)

    def put(blob, L, name, rows, arr):
        lo, hi = L[name]
        blob[rows[0]:rows[1], lo:hi] = arr.astype(np_bf16)

    put(t2, T2_L, "wf1d", (0, DEMB), Wf1[CEMB:, :])
    put(t2, T2_L, "deT", (0, DEMB), de.T)
    put(t2, T2_L, "meb", (0, 100), me)
    t2[:, T2_L["de"][0]:T2_L["de"][0] + DEMB] = de[0:P, :].astype(np_bf16)
    t2[:, T2_L["de"][0] + DEMB:T2_L["de"][0] + 2 * DEMB] = \
        de[P:NDRUG, :].astype(np_bf16)
    ts = np.zeros((P, NTS), np_bf16)
    put(ts, TS_L, "grp_bc", (0, NG),
        np.array([[1.0 if (p // 16) == g else 0.0 for p in range(P)]
                  for g in range(NG)], np.float32))
    put(ts, TS_L, "grp_rd", (0, P),
        np.array([[1.0 if (p // 16) == g else 0.0 for g in range(NG)]
                  for p in range(P)], np.float32))
    sel = np.zeros((NG, NG * P), np.float32)
    for g in range(NG):
        sel[g, g * P:(g + 1) * P] = 1.0
    put(ts, TS_L, "selg", (0, NG), sel)

    put(t3, T3_L, "wf2a", (0, P), Wf2[0:P, :])
    put(t3, T3_L, "wf2b", (0, HID - P), Wf2[P:HID, :])
    w3p = Wf3[:, [1, 2, 3, 4, 5, 6, 7, 8, 0]]
    b3p = bf3[[1, 2, 3, 4, 5, 6, 7, 8, 0]]
    fma = np.concatenate([w3p[0:P, 0:8], np.tile(w3p[0:P, 8:9], (1, 9))], axis=1)
    put(t3, T3_L, "fma", (0, P), fma)
    fmb = np.zeros((P, 17), np.float32)
    fmb[0:HID - P, 0:8] = w3p[P:HID, 0:8]
    fmb[0:HID - P, 8:17] = np.tile(w3p[P:HID, 8:9], (1, 9))
    fmb[P - 1, 0:8] = b3p[0:8]
    fmb[P - 1, 8:17] = b3p[8]
    put(t3, T3_L, "fmb", (0, P), fmb)
    l8 = np.zeros((8, 17), np.float32)
    l8[:, 8:17] = np.triu(np.ones((8, NDOSES), np.float32), 1)
    put(t3, T3_L, "l8", (0, 8), l8)
    put(t3, T3_L, "ones512", (0, 1), np.ones((1, 512), np.float32))

    br = np.zeros((1, NBR), np.float32)
    br[0, BR_L["b1r"][0]:BR_L["b1r"][1]] = b1
    br[0, BR_L["bf1r"][0]:BR_L["bf1r"][1]] = bf1
    br[0, BR_L["onesr"][0]:BR_L["onesr"][1]] = 1.0

    bc = np.zeros((P, NBC), np.float32)
    pp = np.arange(P)
    bc[:, BC_L["qi"][0]] = pp % 16
    bc[:, BC_L["ccl"][0]] = np.where(pp < 100, pp, pp + 28)
    bc[:, BC_L["cch"][0]] = np.where(pp < HID - P, pp + 156, 1000)
    bc[:, BC_L["cdl"][0]] = pp
    bc[:, BC_L["cdh"][0]] = pp + P
    bc[:, BC_L["bf2a"][0]] = bf2[0:P]
    bc[0:HID - P, BC_L["bf2b"][0]] = bf2[P:HID]

    shared = dict(
        blob_t1=np.ascontiguousarray(t1),
        blob_t2=np.ascontiguousarray(t2),
        blob_t3=np.ascontiguousarray(t3),
        blob_ts=np.ascontiguousarray(ts),
        brow=np.ascontiguousarray(br),
        bcol=np.ascontiguousarray(bc),
        # slab layout replicated to all 8 groups on the host (layout-only op)
        csmi=np.ascontiguousarray(
            np.tile((cm | (mi << 7)).reshape(16, SLAB), (NG, 1))),
        dm_map=np.ascontiguousarray(np.tile(dmv.reshape(16, SLAB), (NG, 1))),
    )

    def wrap16(vals):
        # vals [8192] in sample order k (g = k>>10, j = k&1023)
        # -> [128, 64] at [16g + (j & 15), j >> 4]
        v = vals.reshape(NG, GS // 16, 16)        # [g, j_hi, j_lo]
        v = np.transpose(v, (0, 2, 1))            # [g, j_lo, j_hi]
        return np.ascontiguousarray(v.reshape(P, GS // 16))

    in_maps = []
    for c in range(NCORES):
        ic = idx[c * BS:(c + 1) * BS]
        tc_ = tidx[c * BS:(c + 1) * BS]
        m = dict(shared)
        m["u_idx"] = wrap16((ic & (SLAB - 1)).astype(np.uint16))
        m["u_tidx"] = wrap16((tc_ & (SLAB - 1)).astype(np.uint16))
        m["q_idx"] = np.ascontiguousarray(
            (ic >> 14).astype(np_bf16).reshape(NG, GS))
        m["q_tidx"] = np.ascontiguousarray(
            (tc_ >> 14).astype(np_bf16).reshape(NG, GS))
        in_maps.append(m)
    return in_maps


def kernel(**inputs):
    nc = _get_nc()
    in_maps = make_in_maps(inputs)
    last_err = None
    for _attempt in range(3):
        try:
            res = run_bass_kernel_spmd(nc, in_maps, core_ids=list(range(NCORES)))
            return np.concatenate(
                [np.ascontiguousarray(res.results[c]["mu9_s"].T)
                 for c in range(NCORES)], axis=0)
        except Exception as e:  # wedged device sometimes recovers on retry
            last_err = e
    raise last_err


# revision 81
# speedup vs baseline: 1.0588x; 1.0329x over previous
"""Trainium2 Bass kernel for nn_DrugResponsePrior (embedding_lookup).

Spec guarantees: cell_map < 100, is_missing in {0,1}, drug_map < 256.  Each
row's result depends only on the cell state cs = cell_map[idx] +
100*is_missing[idx] (200 states) and dm = drug_map[tidx] (256 drugs).

Fully data-parallel (8 cores x 8192 samples, no collectives).  Per core:
  1. Host bit-packs csmi = cell_map | (is_missing << 7) (u8 - a pure bit
     repack; the state CODE cm + 128*mi is exact in bf16).  csmi/drug_map are
     loaded in a 16-slab SBUF layout (partition 16g+r holds entries
     [r*16384, (r+1)*16384) for every group g).
  2. Two GPSIMD indirect_copy gathers per 4096-sample piece fetch the 16
     slab candidates per sample; a one-hot mask over idx>>14 (grp_bc matmul +
     is_eq) and a group-reduce matmul produce v8 = per-sample code [8, 1024].
  3. Tables built once on device: A = l2n(cell emb) @ Wf1c + bf1 ([200,200])
     and Bd = l2n(drug_emb) @ Wf1d ([256,200]), bf16.
  4. Per 512-sample chunk: a selector matmul broadcasts the chunk's codes to
     [128, 512] PSUM, copied to bf16 SBUF (scalar+vector) so the four DVE
     is_eq one-hot builds run in the 4x mode; bf16 matmuls run the MLP:
     h1 = relu(A^T Sc + Bd^T Sd), h2 = relu(Wf2^T h1 + bf2), fm =
     [fwd(1:9); mu-base] via one packed [.,17] lhsT, softplus (exp+ln) on
     the scalar engine, one L8 matmul accumulates the cumsum into the mu
     rows, and the mu slice is DMA'd out per chunk.
  Chunks are software-pipelined depth 3 so every engine streams without
  gaps (keeps the PE p-state at full clock); the piece-1 lookup resolve is
  emitted mid-chunk-stream so engine FIFOs never head-of-line block on the
  (slow, ~14.5us each) gpsimd gathers.

All params ride in a few packed blob tensors ([128, N] with large
contiguous partition lines, ordered by need-time across the two HWDGE
queue groups) - the baseline's ~16k small DMA descriptors and its
stride-0 replicating slab reads were the main setup bottlenecks.

All reference math runs on device; the host only reshapes/transposes/casts
inputs, bit-packs the two sub-byte index tables, and slices idx/tidx (pure
index arithmetic: & 16383, >> 14).
"""
import os
import sys

if "/opt/trn_rl_repo" not in sys.path:
    sys.path.insert(0, "/opt/trn_rl_repo")

# Neither CoreSim nor this neuronxcc's activation tables implement Softplus;
# softplus is computed as ln(1 + exp(x)) in two scalar passes.
USE_SOFTPLUS = False

import numpy as np
import ml_dtypes

import concourse.bass as bass
import concourse.bass_isa as bass_isa
import concourse.mybir as mybir
import concourse.tile as tile
from concourse.bass_utils import run_bass_kernel_spmd

f32 = mybir.dt.float32
bf16 = mybir.dt.bfloat16
u16 = mybir.dt.uint16
u8 = mybir.dt.uint8
np_bf16 = ml_dtypes.bfloat16

B = 65536
R = 262144
NDRUG = 256
NFEAT = 1024
CEMB = 1024
DEMB = 128
HID = 200
NDOSES = 9
NCORES = 8

BS = B // NCORES            # 8192 samples per core
P = 128
NG = 8                      # gpsimd groups (16 partitions each)
GS = BS // NG               # 1024 samples per group
SLAB = R // 16              # 16384 entries per slab partition
EPS = 1e-12

_NC_CACHE = {}

# ---------------- packed blob layouts (host & device share these) ----------
def _layout(specs):
    out, off = {}, 0
    for name, n in specs:
        out[name] = (off, off + n)
        off += n
    return out, off

# bf16 param blobs, split by when the device needs them (3 staged DMAs)
T1_L, NT1 = _layout([
    ("w1", 8 * CEMB),        # 8 k-tiles [128, 1024]
    ("cft", 8 * 100),        # 8 k-tiles [128, 100]
])
T2_L, NT2 = _layout([
    ("wf1c", 8 * HID),       # 8 k-tiles [128, 200]
    ("wf1d", HID),           # [128, 200]
    ("deT", NDRUG),          # [128, 256] drug_emb^T
    ("meb", CEMB),           # rows 0:100 = missing_emb
    ("de", 2 * DEMB),        # drug_emb [256, 128] as two [128, 128] tiles
])
# tiny early-loaded bf16 constants (lookup resolve must not wait for t2)
TS_L, NTS = _layout([
    ("grp_bc", P),           # rows 0:8: [g, p] = (p//16 == g)
    ("grp_rd", NG),          # [p, g] = (p//16 == g)
    ("selg", NG * P),        # block g: [g', p] = (g' == g)  (bcast lhsT)
])
T3_L, NT3 = _layout([
    ("wf2a", HID),           # Wf2[0:128, :]
    ("wf2b", HID),           # rows 0:72 = Wf2[128:200, :]
    ("fma", 17),             # [Wf3p[0:128, 0:8] | tile(col base, 9)]
    ("fmb", 17),             # rows 0:72 = Wf3p[128:200]; row 127 = biases
    ("l8", 17),              # rows 0:8: cols 0:8 zero, cols 8:17 (k < o)
    ("ones512", 512),        # row 0 = 1.0 (DMA'd to h2s1_st row 127)
])
# single-row f32 tensor (bias rows for matmul rhs; base partition 0)
BR_L, NBR = _layout([
    ("b1r", CEMB),
    ("bf1r", HID),
    ("onesr", P),
])
# per-partition f32 columns [128, NBC]
BC_L, NBC = _layout([
    ("qi", 1),               # p % 16
    ("ccl", 1),              # cs code, one-hot block lo
    ("cch", 1),              # cs code, block hi
    ("cdl", 1),              # dm code lo
    ("cdh", 1),              # dm code hi
    ("bf2a", 1),
    ("bf2b", 1),
])


def _split_sync_waits(nc, limit=1):
    """The walrus accepts at most one sync-wait per instruction; hoist excess
    waits onto same-engine NoOps inserted just before."""
    ctr = 0
    for bb in nc.main_func.blocks:
        new_list = []
        for inst in bb.instructions:
            si = inst.sync_info
            if si is not None and si.on_wait and len(si.on_wait) > limit:
                waits = list(si.on_wait)
                head, tail = waits[:-limit], waits[-limit:]
                for j in range(0, len(head), limit):
                    nop = mybir.InstNoOp(name=f"waitnop-{ctr}", engine=inst.engine)
                    ctr += 1
                    nop.sync_info = mybir.SyncInfo(
                        on_wait=list(head[j : j + limit]), on_update=[]
                    )
                    new_list.append(nop)
                inst.sync_info = mybir.SyncInfo(
                    on_wait=list(tail),
                    on_update=list(si.on_update) if si.on_update else [],
                )
            new_list.append(inst)
        bb.instructions[:] = new_list
    return nc


def build_nc(split_waits=True):
    nc = bass.Bass(num_devices=NCORES)
    AF = mybir.ActivationFunctionType
    ALU = mybir.AluOpType

    # ---------------- kernel I/O ----------------
    blob_t1 = nc.dram_tensor("blob_t1", [P, NT1], bf16, kind="ExternalInput")
    blob_t2 = nc.dram_tensor("blob_t2", [P, NT2], bf16, kind="ExternalInput")
    blob_t3 = nc.dram_tensor("blob_t3", [P, NT3], bf16, kind="ExternalInput")
    blob_ts = nc.dram_tensor("blob_ts", [P, NTS], bf16, kind="ExternalInput")
    brow = nc.dram_tensor("brow", [1, NBR], f32, kind="ExternalInput")
    bcol = nc.dram_tensor("bcol", [P, NBC], f32, kind="ExternalInput")
    # host pre-replicates the 16-slab tables to all 8 groups (contiguous
    # DMA: ~266 GB/s vs ~80 GB/s for a stride-0 replicating read)
    csmi = nc.dram_tensor("csmi", [P, SLAB], u8, kind="ExternalInput")
    dm_map = nc.dram_tensor("dm_map", [P, SLAB], u8, kind="ExternalInput")
    u_idx = nc.dram_tensor("u_idx", [P, GS // 16], u16, kind="ExternalInput")
    u_tidx = nc.dram_tensor("u_tidx", [P, GS // 16], u16, kind="ExternalInput")
    q_idx = nc.dram_tensor("q_idx", [NG, GS], bf16, kind="ExternalInput")
    q_tidx = nc.dram_tensor("q_tidx", [NG, GS], bf16, kind="ExternalInput")
    mu9_s = nc.dram_tensor("mu9_s", [NDOSES, BS], f32, kind="ExternalOutput")

    with tile.TileContext(nc) as tc, \
            tc.tile_pool(name="sbw", bufs=1) as sbw, \
            tc.tile_pool(name="sb", bufs=1) as sb:

        # ---- setup DMAs ----
        # queue A (sync -> DMA engines 0-7): index tensors + slabs
        # queue B (scalar -> DMA engines 8-15): param blobs, staged
        u_idx_sb = sb.tile([P, GS // 16], u16)
        u_tidx_sb = sb.tile([P, GS // 16], u16)
        q_idx_sb = sb.tile([NG, GS], bf16)
        q_tidx_sb = sb.tile([NG, GS], bf16)
        bc_sb = sb.tile([P, NBC], f32)
        nc.sync.dma_start(out=bc_sb[:], in_=bcol[:])
        nc.sync.dma_start(out=u_idx_sb[:], in_=u_idx[:])
        nc.sync.dma_start(out=u_tidx_sb[:], in_=u_tidx[:])
        nc.sync.dma_start(out=q_idx_sb[:], in_=q_idx[:])
        nc.sync.dma_start(out=q_tidx_sb[:], in_=q_tidx[:])
        cs_slab = sbw.tile([P, SLAB], u8)
        dm_slab = sbw.tile([P, SLAB], u8)
        t1_sb = sbw.tile([P, NT1], bf16)
        # A queue: cs table first (gathers gate everything downstream)
        nc.sync.dma_start(out=cs_slab[:], in_=csmi[:])
        # B queue: tiny consts, then dm table
        br_sb = sb.tile([1, NBR], f32)
        nc.scalar.dma_start(out=br_sb[:], in_=brow[:])
        ts_sb = sb.tile([P, NTS], bf16)
        nc.scalar.dma_start(out=ts_sb[:], in_=blob_ts[:])
        nc.scalar.dma_start(out=dm_slab[:], in_=dm_map[:])
        # t1 streamed per k-tile, alternating queues, so the P100 matmuls
        # start as soon as each tile lands instead of after the whole blob
        for kt in range(8):
            eng = nc.sync if kt % 2 == 0 else nc.scalar
            w0, _ = T1_L["w1"]
            c0, _ = T1_L["cft"]
            eng.dma_start(out=t1_sb[:, w0 + kt * CEMB:w0 + (kt + 1) * CEMB],
                          in_=blob_t1[:, w0 + kt * CEMB:w0 + (kt + 1) * CEMB])
            eng.dma_start(out=t1_sb[:, c0 + kt * 100:c0 + (kt + 1) * 100],
                          in_=blob_t1[:, c0 + kt * 100:c0 + (kt + 1) * 100])
        t2_sb = sbw.tile([P, NT2], bf16)
        nc.scalar.dma_start(out=t2_sb[:], in_=blob_t2[:])
        t3_sb = sbw.tile([P, NT3], bf16)
        nc.scalar.dma_start(out=t3_sb[:], in_=blob_t3[:])

        # blob views
        me_sb = t2_sb[0:100, T2_L["meb"][0]:T2_L["meb"][1]]
        b1_row = br_sb[:, BR_L["b1r"][0]:BR_L["b1r"][1]]
        bf1_row = br_sb[:, BR_L["bf1r"][0]:BR_L["bf1r"][1]]
        ones100 = br_sb[:, BR_L["onesr"][0]:BR_L["onesr"][0] + 100]
        ones128 = br_sb[:, BR_L["onesr"][0]:BR_L["onesr"][1]]
        de0 = t2_sb[:, T2_L["de"][0]:T2_L["de"][0] + DEMB]
        de1 = t2_sb[:, T2_L["de"][0] + DEMB:T2_L["de"][0] + 2 * DEMB]
        qi_c = bc_sb[:, BC_L["qi"][0]:BC_L["qi"][1]]
        ccl_c = bc_sb[:, BC_L["ccl"][0]:BC_L["ccl"][1]]
        cch_c = bc_sb[:, BC_L["cch"][0]:BC_L["cch"][1]]
        cdl_c = bc_sb[:, BC_L["cdl"][0]:BC_L["cdl"][1]]
        cdh_c = bc_sb[:, BC_L["cdh"][0]:BC_L["cdh"][1]]
        bf2a_c = bc_sb[:, BC_L["bf2a"][0]:BC_L["bf2a"][1]]
        bf2b_c = bc_sb[0:72, BC_L["bf2b"][0]:BC_L["bf2b"][1]]
        w1_kt = [t1_sb[:, T1_L["w1"][0] + k * CEMB:T1_L["w1"][0] + (k + 1) * CEMB]
                 for k in range(8)]
        cft_kt = [t1_sb[:, T1_L["cft"][0] + k * 100:T1_L["cft"][0] + (k + 1) * 100]
                  for k in range(8)]
        wf1c_kt = [t2_sb[:, T2_L["wf1c"][0] + k * HID:T2_L["wf1c"][0] + (k + 1) * HID]
                   for k in range(8)]
        wf1d_sb = t2_sb[:, T2_L["wf1d"][0]:T2_L["wf1d"][1]]
        deT_sb = t2_sb[:, T2_L["deT"][0]:T2_L["deT"][1]]
        grp_bc = ts_sb[0:NG, TS_L["grp_bc"][0]:TS_L["grp_bc"][1]]
        grp_rd = ts_sb[:, TS_L["grp_rd"][0]:TS_L["grp_rd"][1]]
        selg = [ts_sb[0:NG, TS_L["selg"][0] + g * P:TS_L["selg"][0] + (g + 1) * P]
                for g in range(NG)]
        wf2a = t3_sb[:, T3_L["wf2a"][0]:T3_L["wf2a"][1]]
        wf2b = t3_sb[0:72, T3_L["wf2b"][0]:T3_L["wf2b"][1]]
        fma = t3_sb[:, T3_L["fma"][0]:T3_L["fma"][1]]
        fmb = t3_sb[:, T3_L["fmb"][0]:T3_L["fmb"][1]]
        l8_sb = t3_sb[0:8, T3_L["l8"][0]:T3_L["l8"][1]]



        # lookup state (lives across the table scope and the chunk scope)
        g_cs = sb.tile([P, GS], u8)
        g_dm = sb.tile([P, GS], u8)
        v8_cs = sb.tile([NG, GS], bf16)
        v8_dm = sb.tile([NG, GS], bf16)

        def emit_resolve(t, mk_ps, names=("c", "d")):
            jsl = slice(t * 512, (t + 1) * 512)
            for (gt, qt, v8t, nm) in ((g_cs, q_idx_sb, v8_cs, "c"),
                                      (g_dm, q_tidx_sb, v8_dm, "d")):
                if nm not in names:
                    continue
                qb = mk_ps()
                nc.tensor.matmul(out=qb[:], lhsT=grp_bc, rhs=qt[:, jsl],
                                 start=True, stop=True)
                qmask = sb.tile([P, 512], bf16, tag=f"qmask_{nm}",
                                name=f"qmask_{nm}")
                nc.vector.tensor_scalar(
                    out=qmask[:], in0=qb[:], scalar1=qi_c, scalar2=None,
                    op0=ALU.is_equal)
                gf = sb.tile([P, 512], bf16, tag=f"gf_{nm}", name=f"gf_{nm}")
                nc.vector.tensor_copy(out=gf[:], in_=gt[:, jsl])
                nc.vector.tensor_tensor(out=gf[:], in0=gf[:], in1=qmask[:],
                                        op=ALU.mult)
                vpf = mk_ps()
                nc.tensor.matmul(out=vpf[0:NG, :], lhsT=grp_rd, rhs=gf[:],
                                 start=True, stop=True)
                nc.vector.tensor_copy(out=v8t[:, jsl], in_=vpf[0:NG, :])

        # ======== table construction: A [200,200], Bd [256,200] (bf16) ======
        a_k = []
        bd_k = []
        with (
            tc.tile_pool(name="ps_tb", bufs=1, space="PSUM") as ps_tb,
            tc.tile_pool(name="ps_tr", bufs=3, space="PSUM") as ps_tr,
            tc.tile_pool(name="sbt", bufs=1) as sbt,
        ):
            from concourse.masks import make_identity
            ident = sbt.tile([P, P], bf16)
            make_identity(nc, ident[:])

            # P100 = relu(cf @ W1 + b1)  [100, 1024]; kt-major so each matmul
            # runs as soon as its streamed k-tile DMA lands
            p_sb = sbt.tile([100, CEMB], bf16)
            pps = [ps_tb.tile([100, 512], f32, tag=f"pshard{nh}",
                              name=f"pps{nh}") for nh in range(2)]
            for kt in range(8):
                for nh in range(2):
                    nc.tensor.matmul(
                        out=pps[nh][:], lhsT=cft_kt[kt],
                        rhs=w1_kt[kt][:, nh * 512:(nh + 1) * 512],
                        start=(kt == 0), stop=False)
            for nh in range(2):
                nc.tensor.matmul(
                    out=pps[nh][:], lhsT=ones100,
                    rhs=b1_row[:, nh * 512:(nh + 1) * 512], start=False, stop=True)
                nc.scalar.activation(
                    out=p_sb[:, nh * 512:(nh + 1) * 512], in_=pps[nh][:],
                    func=AF.Relu)

            # l2 norm scales for present / missing rows
            sq = sbt.tile([100, CEMB], f32)
            ssp = sbt.tile([100, 1], f32)
            ssm = sbt.tile([100, 1], f32)
            nc.scalar.activation(out=sq[:], in_=p_sb[:], func=AF.Square)
            nc.vector.reduce_sum(out=ssp[:], in_=sq[:], axis=mybir.AxisListType.X)
            nc.scalar.activation(out=sq[:], in_=me_sb, func=AF.Square)
            nc.vector.reduce_sum(out=ssm[:], in_=sq[:], axis=mybir.AxisListType.X)
            for ss in (ssp, ssm):
                nc.scalar.activation(out=ss[:], in_=ss[:], func=AF.Sqrt)
                nc.vector.tensor_scalar_max(out=ss[:], in0=ss[:], scalar1=EPS)
                nc.vector.reciprocal(out=ss[:], in_=ss[:])
            nc.vector.tensor_scalar_mul(out=p_sb[:], in0=p_sb[:], scalar1=ssp[:])
            nc.vector.tensor_scalar_mul(out=me_sb, in0=me_sb, scalar1=ssm[:])

            # CnT k-tiles [128, 200] bf16 (cols: 100 present + 100 missing)
            cnt_kt = []
            for kt in range(8):
                t = sbt.tile([P, 2 * 100], bf16, tag=f"cnt_{kt}")
                for (src, co) in ((p_sb[:], 0), (me_sb, 100)):
                    tp = ps_tr.tile([P, 100], bf16, tag="tr")
                    nc.tensor.transpose(
                        out=tp[:], in_=src[:, kt * P:(kt + 1) * P],
                        identity=ident[:100, :100])
                    nc.vector.tensor_copy(out=t[:, co:co + 100], in_=tp[:])
                cnt_kt.append(t)

            # A tiles (states on partitions): a_k[0] [128, 200], a_k[1] [72, 200]
            for (mt, msl) in ((0, slice(0, P)), (1, slice(P, HID))):
                mm = msl.stop - msl.start
                aps = ps_tb.tile([P, HID], f32, tag="a")
                for kt in range(8):
                    nc.tensor.matmul(
                        out=aps[:mm, :], lhsT=cnt_kt[kt][:, msl],
                        rhs=wf1c_kt[kt], start=(kt == 0), stop=False)
                nc.tensor.matmul(
                    out=aps[:mm, :], lhsT=ones128[:, :mm], rhs=bf1_row,
                    start=False, stop=True)
                t = sb.tile([mm, HID], bf16, tag=f"a_{mt}")
                nc.vector.tensor_copy(out=t[:], in_=aps[:mm, :])
                a_k.append(t)

            # drug tiles: per-drug l2 recip + Bd [128, 200] bf16 x2
            for (mt, de_p) in ((0, de0), (1, de1)):
                sqd = sbt.tile([P, DEMB], f32, tag="sqd")
                rd = sbt.tile([P, 1], f32, tag=f"rd_{mt}")
                nc.scalar.activation(out=sqd[:], in_=de_p, func=AF.Square)
                nc.vector.reduce_sum(out=rd[:], in_=sqd[:], axis=mybir.AxisListType.X)
                nc.scalar.activation(out=rd[:], in_=rd[:], func=AF.Sqrt)
                nc.vector.tensor_scalar_max(out=rd[:], in0=rd[:], scalar1=EPS)
                nc.vector.reciprocal(out=rd[:], in_=rd[:])
                bps = ps_tb.tile([P, HID], f32, tag="a")
                nc.tensor.matmul(out=bps[:], lhsT=deT_sb[:, mt * P:(mt + 1) * P],
                                 rhs=wf1d_sb, start=True, stop=True)
                t = sb.tile([P, HID], bf16, tag=f"bd_{mt}")
                nc.vector.tensor_scalar_mul(out=t[:], in0=bps[:], scalar1=rd[:])
                bd_k.append(t)

            # ======== lookup gathers (gpsimd runs these back to back) ========
            for t in range(2):
                nc.gpsimd.indirect_copy(
                    out=g_cs[:, t * 512:(t + 1) * 512].rearrange(
                        "p (n one) -> p n one", one=1),
                    data=cs_slab[:], idxs=u_idx_sb[:, t * 32:(t + 1) * 32],
                    i_know_ap_gather_is_preferred=True)
                nc.gpsimd.indirect_copy(
                    out=g_dm[:, t * 512:(t + 1) * 512].rearrange(
                        "p (n one) -> p n one", one=1),
                    data=dm_slab[:], idxs=u_tidx_sb[:, t * 32:(t + 1) * 32],
                    i_know_ap_gather_is_preferred=True)
            # piece 0 resolved here; piece 1 resolved mid-chunk-stream so the
            # engine FIFOs don't head-of-line block on its gathers
            emit_resolve(0, lambda: ps_tb.tile([P, 512], f32, tag="pqb",
                                               name="pqb"))

        # ======== per-chunk pipeline ========
        chunks = [(g, pc) for pc in range(2) for g in range(NG)]
        NCH = len(chunks)

        with (
            tc.tile_pool(name="ps_h1", bufs=1, space="PSUM") as ps_h1,
            tc.tile_pool(name="ps_h2", bufs=1, space="PSUM") as ps_h2,
            tc.tile_pool(name="ps_fm", bufs=2, space="PSUM") as ps_fm,
            tc.tile_pool(name="ps_qb", bufs=1, space="PSUM") as ps_qb,
            tc.tile_pool(name="sbc", bufs=2) as sbc,
        ):
            bc_of, oh_of, h1ps_of, h1s_of, h2ps_of, h2s_of, fm_of, spb_of = \
                {}, {}, {}, {}, {}, {}, {}, {}

            # manually double-buffered h2 relu tiles (h2s1 rows 72:127 zero,
            # row 127 ones so fmb's row 127 supplies the biases); chunks
            # alternate by parity so the pad init stays on one tile handle
            h2s0_ab = [sb.tile([P, 512], bf16, tag=f"h2s0_{b}",
                               name=f"h2s0_{b}") for b in range(2)]
            h2s1_ab = [sb.tile([P, 512], bf16, tag=f"h2s1_{b}",
                               name=f"h2s1_{b}") for b in range(2)]
            for t in h2s1_ab:
                nc.vector.memset(t[:], 0.0)
                nc.sync.dma_start(
                    out=t[P - 1:P, :],
                    in_=blob_t3[0:1, T3_L["ones512"][0]:T3_L["ones512"][1]])

            def emit_qb(i):
                # broadcast codes of chunk i's group to all 128 partitions
                g, pc = chunks[i]
                jsl = slice(pc * 512, (pc + 1) * 512)
                qbc = ps_qb.tile([P, 512], f32, tag="qbc")
                qbd = ps_qb.tile([P, 512], f32, tag="qbd")
                nc.tensor.matmul(out=qbc[:], lhsT=selg[g], rhs=v8_cs[:, jsl],
                                 start=True, stop=True)
                nc.tensor.matmul(out=qbd[:], lhsT=selg[g], rhs=v8_dm[:, jsl],
                                 start=True, stop=True)
                bc_of[i] = (qbc, qbd)

            def emit_bcopy(i):
                # bf16 SBUF copies so the is_eq runs in the DVE 4x mode;
                # both on scalar (DVE is the tighter engine)
                qbc, qbd = bc_of.pop(i)
                bcc = sbc.tile([P, 512], bf16, tag="bcc")
                bcd = sbc.tile([P, 512], bf16, tag="bcd")
                nc.scalar.activation(out=bcc[:], in_=qbc[:], func=AF.Copy)
                nc.vector.tensor_copy(out=bcd[:], in_=qbd[:])
                bc_of[i] = (bcc, bcd)

            def emit_onehot(i):
                eng = nc.vector
                bcc, bcd = bc_of.pop(i)
                sc2 = sbc.tile([P, 1024], bf16, tag="sc2")
                sd2 = sbc.tile([P, 1024], bf16, tag="sd2")
                for (oh, bc, cl, ch_) in ((sc2, bcc, ccl_c, cch_c),
                                          (sd2, bcd, cdl_c, cdh_c)):
                    eng.tensor_scalar(
                        out=oh[:, 0:512], in0=bc[:], scalar1=cl, scalar2=None,
                        op0=ALU.is_equal)
                    eng.tensor_scalar(
                        out=oh[:, 512:1024], in0=bc[:], scalar1=ch_, scalar2=None,
                        op0=ALU.is_equal)
                oh_of[i] = (sc2, sd2)

            def emit_h1(i):
                sc2, sd2 = oh_of.pop(i)
                hps = []
                for (mt, msl) in ((0, slice(0, P)), (1, slice(P, HID))):
                    mm = msl.stop - msl.start
                    hp = ps_h1.tile([mm, 512], f32, tag=f"h1_{mt}")
                    nc.tensor.matmul(out=hp[:], lhsT=a_k[0][:, msl],
                                     rhs=sc2[:, 0:512], start=True, stop=False)
                    nc.tensor.matmul(out=hp[:], lhsT=a_k[1][:, msl],
                                     rhs=sc2[0:HID - P, 512:1024],
                                     start=False, stop=False)
                    nc.tensor.matmul(out=hp[:], lhsT=bd_k[0][:, msl],
                                     rhs=sd2[:, 0:512], start=False, stop=False)
                    nc.tensor.matmul(out=hp[:], lhsT=bd_k[1][:, msl],
                                     rhs=sd2[:, 512:1024], start=False, stop=True)
                    hps.append(hp)
                h1ps_of[i] = hps

            def emit_h1relu(i):
                hps = h1ps_of.pop(i)
                h1s = []
                for mt, hp in enumerate(hps):
                    mm = P if mt == 0 else HID - P
                    hs = sbc.tile([mm, 512], bf16, tag=f"h1s_{mt}")
                    nc.vector.tensor_scalar_max(out=hs[:], in0=hp[:], scalar1=0.0)
                    h1s.append(hs)
                h1s_of[i] = h1s

            def emit_h2(i):
                h1s = h1s_of.pop(i)
                hps = []
                for (mt, msl) in ((0, slice(0, P)), (1, slice(P, HID))):
                    mm = msl.stop - msl.start
                    hp = ps_h2.tile([mm, 512], f32, tag=f"h2_{mt}")
                    nc.tensor.matmul(out=hp[:], lhsT=wf2a[:, msl], rhs=h1s[0][:],
                                     start=True, stop=False)
                    nc.tensor.matmul(out=hp[:], lhsT=wf2b[:, msl], rhs=h1s[1][:],
                                     start=False, stop=True)
                    hps.append(hp)
                h2ps_of[i] = hps

            def emit_h2relu(i):
                hps = h2ps_of.pop(i)
                h2s0 = h2s0_ab[i % 2]
                h2s1 = h2s1_ab[i % 2]
                nc.scalar.activation(out=h2s0[:], in_=hps[0][:], func=AF.Relu,
                                     bias=bf2a_c, scale=1.0)
                nc.scalar.activation(out=h2s1[0:HID - P, :], in_=hps[1][:],
                                     func=AF.Relu, bias=bf2b_c, scale=1.0)
                h2s_of[i] = (h2s0, h2s1)

            def emit_fm(i):
                h2s0, h2s1 = h2s_of.pop(i)
                fm = ps_fm.tile([8 + NDOSES, 512], f32, tag="fm")
                nc.tensor.matmul(out=fm[:], lhsT=fma, rhs=h2s0[:],
                                 start=True, stop=False)
                nc.tensor.matmul(out=fm[:], lhsT=fmb, rhs=h2s1[:],
                                 start=False, stop=True)
                fm_of[i] = fm

            def emit_softplus(i):
                fm = fm_of[i]
                spb = sbc.tile([8, 512], bf16, tag="spb")
                if USE_SOFTPLUS:
                    nc.scalar.activation(out=spb[:], in_=fm[0:8, :],
                                         func=AF.Softplus)
                else:
                    nc.scalar.activation(out=spb[:], in_=fm[0:8, :], func=AF.Exp)
                    nc.scalar.activation(out=spb[:], in_=spb[:], func=AF.Ln,
                                         bias=1.0, scale=1.0)
                spb_of[i] = spb

            def emit_l8(i):
                fm = fm_of[i]
                spb = spb_of.pop(i)
                nc.tensor.matmul(out=fm[:], lhsT=l8_sb, rhs=spb[:],
                                 start=False, stop=True, skip_group_check=True)

            def emit_mucopy(i):
                # rows 0:8 = spent f9 junk (not stored); rows 8:17 = mu
                g, pc = chunks[i]
                fm = fm_of.pop(i)
                n0 = g * GS + pc * 512
                muc = sbc.tile([8 + NDOSES, 512], f32, tag="muc")
                nc.vector.tensor_copy(out=muc[:], in_=fm[:])
                nc.sync.dma_start(out=mu9_s[:, n0:n0 + 512],
                                  in_=muc[8:8 + NDOSES, :])

            # prologue
            emit_qb(0)
            emit_bcopy(0)
            emit_onehot(0)

            mk_prs = lambda: ps_h1.tile([P, 512], f32, tag="h1_0", name="prs")
            # fm is deferred a FULL iteration behind h2 so the scalar h2-relu
            # never stalls the tensor stream (which would also reset the PE
            # p-state ramp); pipeline depth 4.
            for i in range(NCH):
                emit_h1(i)
                emit_h1relu(i)
                if i >= 1:
                    emit_h2(i - 1)
                    emit_h2relu(i - 1)
                if i == NCH // 2 - 2:
                    # resolve piece 1 cs (its gather lands around now); the
                    # h1_0 bank's next chunk use is pc-1 (needs this anyway)
                    emit_resolve(1, mk_prs, names=("c",))
                if i + 1 < NCH and i != NCH // 2 - 1:
                    emit_qb(i + 1)
                    emit_bcopy(i + 1)
                    emit_onehot(i + 1)
                if i >= 2:
                    emit_fm(i - 2)
                    emit_softplus(i - 2)
                if i == NCH // 2 - 1:
                    # piece-1 dm resolve late in the iteration, then the
                    # deferred qb for the first pc-1 chunk
                    emit_resolve(1, mk_prs, names=("d",))
                    emit_qb(i + 1)
                    emit_bcopy(i + 1)
                    emit_onehot(i + 1)
                if i >= 3:
                    emit_l8(i - 3)
                    emit_mucopy(i - 3)
            emit_h2(NCH - 1)
            emit_h2relu(NCH - 1)
            emit_fm(NCH - 2)
            emit_softplus(NCH - 2)
            emit_l8(NCH - 3)
            emit_mucopy(NCH - 3)
            emit_fm(NCH - 1)
            emit_softplus(NCH - 1)
            emit_l8(NCH - 2)
            emit_mucopy(NCH - 2)
            emit_l8(NCH - 1)
            emit_mucopy(NCH - 1)

    return _split_sync_waits(nc) if split_waits else nc


def _get_nc():
    if "nc" not in _NC_CACHE:
        _NC_CACHE["nc"] = build_nc()
    return _NC_CACHE["nc"]


def make_in_maps(inputs):
    idx = np.asarray(inputs["idx"], np.int64)
    tidx = np.asarray(inputs["tidx"], np.int64)
    cm = np.asarray(inputs["cell_map"]).astype(np.uint8)
    mi = np.asarray(inputs["is_missing"]).astype(np.uint8)
    dmv = np.asarray(inputs["drug_map"]).astype(np.uint8)
    cf = np.asarray(inputs["cell_features"], np.float32)
    me = np.asarray(inputs["missing_emb"], np.float32)
    de = np.asarray(inputs["drug_emb"], np.float32)
    W1 = np.asarray(inputs["W1"], np.float32)
    Wf1 = np.asarray(inputs["Wf1"], np.float32)
    Wf2 = np.asarray(inputs["Wf2"], np.float32)
    Wf3 = np.asarray(inputs["Wf3"], np.float32)
    b1 = np.asarray(inputs["b1"], np.float32)
    bf1 = np.asarray(inputs["bf1"], np.float32)
    bf2 = np.asarray(inputs["bf2"], np.float32)
    bf3 = np.asarray(inputs["bf3"], np.float32)

    # ---- bf16 blobs ----
    t1 = np.zeros((P, NT1), np_bf16)
    t2 = np.zeros((P, NT2), np_bf16)
    t3 = np.zeros((P, NT3), np_bf16)

    for kt in range(8):
        t1[:, T1_L["w1"][0] + kt * CEMB:T1_L["w1"][0] + (kt + 1) * CEMB] = \
            W1[kt * P:(kt + 1) * P, :].astype(np_bf16)
        t1[:, T1_L["cft"][0] + kt * 100:T1_L["cft"][0] + (kt + 1) * 100] = \
            cf[:100, kt * P:(kt + 1) * P].T.astype(np_bf16)
        t2[:, T2_L["wf1c"][0] + kt * HID:T2_L["wf1c"][0] + (kt + 1) * HID] = \
            Wf1[kt * P:(kt + 1) * P, :].astype(np_bf16)

    def put(blob, L, name, rows, arr):
        lo, hi = L[name]
        blob[rows[0]:rows[1], lo:hi] = arr.astype(np_bf16)

    put(t2, T2_L, "wf1d", (0, DEMB), Wf1[CEMB:, :])
    put(t2, T2_L, "deT", (0, DEMB), de.T)
    put(t2, T2_L, "meb", (0, 100), me)
    t2[:, T2_L["de"][0]:T2_L["de"][0] + DEMB] = de[0:P, :].astype(np_bf16)
    t2[:, T2_L["de"][0] + DEMB:T2_L["de"][0] + 2 * DEMB] = \
        de[P:NDRUG, :].astype(np_bf16)
    ts = np.zeros((P, NTS), np_bf16)
    put(ts, TS_L, "grp_bc", (0, NG),
        np.array([[1.0 if (p // 16) == g else 0.0 for p in range(P)]
                  for g in range(NG)], np.float32))
    put(ts, TS_L, "grp_rd", (0, P),
        np.array([[1.0 if (p // 16) == g else 0.0 for g in range(NG)]
                  for p in range(P)], np.float32))
    sel = np.zeros((NG, NG * P), np.float32)
    for g in range(NG):
        sel[g, g * P:(g + 1) * P] = 1.0
    put(ts, TS_L, "selg", (0, NG), sel)

    put(t3, T3_L, "wf2a", (0, P), Wf2[0:P, :])
    put(t3, T3_L, "wf2b", (0, HID - P), Wf2[P:HID, :])
    w3p = Wf3[:, [1, 2, 3, 4, 5, 6, 7, 8, 0]]
    b3p = bf3[[1, 2, 3, 4, 5, 6, 7, 8, 0]]
    fma = np.concatenate([w3p[0:P, 0:8], np.tile(w3p[0:P, 8:9], (1, 9))], axis=1)
    put(t3, T3_L, "fma", (0, P), fma)
    fmb = np.zeros((P, 17), np.float32)
    fmb[0:HID - P, 0:8] = w3p[P:HID, 0:8]
    fmb[0:HID - P, 8:17] = np.tile(w3p[P:HID, 8:9], (1, 9))
    fmb[P - 1, 0:8] = b3p[0:8]
    fmb[P - 1, 8:17] = b3p[8]
    put(t3, T3_L, "fmb", (0, P), fmb)
    l8 = np.zeros((8, 17), np.float32)
    l8[:, 8:17] = np.triu(np.ones((8, NDOSES), np.float32), 1)
    put(t3, T3_L, "l8", (0, 8), l8)
    put(t3, T3_L, "ones512", (0, 1), np.ones((1, 512), np.float32))

    br = np.zeros((1, NBR), np.float32)
    br[0, BR_L["b1r"][0]:BR_L["b1r"][1]] = b1
    br[0, BR_L["bf1r"][0]:BR_L["bf1r"][1]] = bf1
    br[0, BR_L["onesr"][0]:BR_L["onesr"][1]] = 1.0

    bc = np.zeros((P, NBC), np.float32)
    pp = np.arange(P)
    bc[:, BC_L["qi"][0]] = pp % 16
    bc[:, BC_L["ccl"][0]] = np.where(pp < 100, pp, pp + 28)
    bc[:, BC_L["cch"][0]] = np.where(pp < HID - P, pp + 156, 1000)
    bc[:, BC_L["cdl"][0]] = pp
    bc[:, BC_L["cdh"][0]] = pp + P
    bc[:, BC_L["bf2a"][0]] = bf2[0:P]
    bc[0:HID - P, BC_L["bf2b"][0]] = bf2[P:HID]

    shared = dict(
        blob_t1=np.ascontiguousarray(t1),
        blob_t2=np.ascontiguousarray(t2),
        blob_t3=np.ascontiguousarray(t3),
        blob_ts=np.ascontiguousarray(ts),
        brow=np.ascontiguousarray(br),
        bcol=np.ascontiguousarray(bc),
        # slab layout replicated to all 8 groups on the host (layout-only op)
        csmi=np.ascontiguousarray(
            np.tile((cm | (mi << 7)).reshape(16, SLAB), (NG, 1))),
        dm_map=np.ascontiguousarray(np.tile(dmv.reshape(16, SLAB), (NG, 1))),
    )

    def wrap16(vals):
        # vals [8192] in sample order k (g = k>>10, j = k&1023)
        # -> [128, 64] at [16g + (j & 15), j >> 4]
        v = vals.reshape(NG, GS // 16, 16)        # [g, j_hi, j_lo]
        v = np.transpose(v, (0, 2, 1))            # [g, j_lo, j_hi]
        return np.ascontiguousarray(v.reshape(P, GS // 16))

    in_maps = []
    for c in range(NCORES):
        ic = idx[c * BS:(c + 1) * BS]
        tc_ = tidx[c * BS:(c + 1) * BS]
        m = dict(shared)
        m["u_idx"] = wrap16((ic & (SLAB - 1)).astype(np.uint16))
        m["u_tidx"] = wrap16((tc_ & (SLAB - 1)).astype(np.uint16))
        m["q_idx"] = np.ascontiguousarray(
            (ic >> 14).astype(np_bf16).reshape(NG, GS))
        m["q_tidx"] = np.ascontiguousarray(
            (tc_ >> 14).astype(np_bf16).reshape(NG, GS))
        in_maps.append(m)
    return in_maps


def kernel(**inputs):
    nc = _get_nc()
    in_maps = make_in_maps(inputs)
    last_err = None
    for _attempt in range(3):
        try:
            res = run_bass_kernel_spmd(nc, in_maps, core_ids=list(range(NCORES)))
            return np.concatenate(
                [np.ascontiguousarray(res.results[c]["mu9_s"].T)
                 for c in range(NCORES)], axis=0)
        except Exception as e:  # wedged device sometimes recovers on retry
            last_err = e
    raise last_err
